# revision 1
# baseline (speedup 1.0000x reference)
"""MultiHeadAttention + RoPE kernel for 8 Trainium2 NeuronCores.

Sharding: core c in 0..7 -> batch b = c//4, head-group hg = c%4 (4 heads
each).  Each core computes its 4 heads' attention for its batch and a
partial output projection y_part = out_heads @ wo[head rows]; the host
sums the 4 partials per batch and adds bo.

Per-core dataflow (all matmuls in float32r = TF32-like, 1 cyc/row):
  - host passes xT = x[b].T so the contraction dim D is on partitions
  - QT/KT computed transposed [depth, S]; bias added during PSUM evac
    (per-partition tensor_scalar), RoPE via a signed-permutation matmul
    (rot) + cos/sin elementwise combines
  - V computed natural [S, depth] with bias via a K=1 ones matmul;
    a ones column is appended -> V' [S, 65]
  - scores computed transposed: matmul(lhsT=KT tile, rhs=QT)
    -> ST [128 keys, S queries]; exp via ACT (scale=1/8 folded in)
  - PV: matmul(lhsT=V'_tile, rhs=expST) accumulates out'T [65, S];
    row 64 is the softmax denominator (from the ones column)
  - normalize with gpsimd partition-broadcast + fast reciprocal
  - O-proj: matmul(lhsT=outT[:, h, q-tile], rhs=wo[:, h, :]) with K=64
    accumulation over the 4 local heads
"""

import numpy as np

import concourse.bacc as bacc
import concourse.mybir as mybir
from concourse.tile import TileContext

try:  # persistent XLA compile cache: repeat processes skip the ~4min compile
    import jax as _jax
    _jax.config.update("jax_compilation_cache_dir", "/tmp/jax_comp_cache")
    _jax.config.update("jax_persistent_cache_min_compile_time_secs", 1.0)
except Exception:
    pass

B, S, DM, H, DH = 2, 2048, 1024, 16, 64
NCORES = 8
HL = 4                # heads per core
DHL = HL * DH         # 256
KCH = DM // 128       # 8 k-chunks of the model-dim contraction
SKT = S // 128        # 16 key tiles
NQ = S // 512         # 4 query chunks of 512 (PSUM bank limit)
QT_TILES = DHL // 128  # 2 m-tiles for the Q/K projections
QB = 1024              # phase-B q block
NQB = S // QB

F32 = mybir.dt.float32
F32R = mybir.dt.float32r
EXP = mybir.ActivationFunctionType.Exp
COPY = mybir.ActivationFunctionType.Copy
ADD = mybir.AluOpType.add
MULT = mybir.AluOpType.mult

_CACHE = {}


def _build_nc(exp_bufs=3, ld_bufs=3, st_bufs=2, pv_bufs=1,
              y_bufs=4, yps_bufs=2):
    nc = bacc.Bacc()
    xT = nc.dram_tensor("xT", [DM, S], F32R, kind="ExternalInput")
    wq = nc.dram_tensor("wq", [DM, DHL], F32R, kind="ExternalInput")
    wk = nc.dram_tensor("wk", [DM, DHL], F32R, kind="ExternalInput")
    wv = nc.dram_tensor("wv", [DM, DHL], F32R, kind="ExternalInput")
    wo = nc.dram_tensor("wo", [128, QT_TILES, DM], F32R, kind="ExternalInput")
    bq = nc.dram_tensor("bq", [128, QT_TILES], F32, kind="ExternalInput")
    bk = nc.dram_tensor("bk", [128, QT_TILES], F32, kind="ExternalInput")
    bv = nc.dram_tensor("bv", [1, DHL], F32R, kind="ExternalInput")
    cosT = nc.dram_tensor("cosT", [128, S], F32, kind="ExternalInput")
    sinT = nc.dram_tensor("sinT", [128, S], F32, kind="ExternalInput")
    prot = nc.dram_tensor("prot", [128, 128], F32R, kind="ExternalInput")
    y = nc.dram_tensor("y", [S, DM], F32, kind="ExternalOutput")

    with TileContext(nc) as tc:
        with tc.tile_pool(name="p0", bufs=1) as p0:
            qrope_r = p0.tile([128, QT_TILES, S], F32R)
            krope_r = p0.tile([128, QT_TILES, S], F32R)
            v_r = p0.tile([128, SKT, HL, DH + 1], F32R)
            prot_r = p0.tile([128, 128], F32R)
            ones_row_r = p0.tile([1, 128], F32R)
            ones_col_f = p0.tile([128, 1], F32)
            bv_r = p0.tile([1, DHL], F32R)
            nc.vector.memset(ones_col_f[:], 1.0)

            # ================= PHASE A =================
            with (
                tc.tile_pool(name="pa", bufs=1) as pa,
                tc.tile_pool(name="pa_ld", bufs=ld_bufs) as pa_ld,
                tc.tile_pool(name="pa_w", bufs=3) as pa_w,
                tc.tile_pool(name="pa_t", bufs=2) as pa_t,
                tc.tile_pool(name="ps_a", bufs=1, space="PSUM") as ps_a,
            ):
                cos_sb = pa.tile([128, S], F32)
                sin_sb = pa.tile([128, S], F32)
                nc.sync.dma_start(cos_sb[:], cosT[:, :])
                nc.sync.dma_start(sin_sb[:], sinT[:, :])
                nc.sync.dma_start(prot_r[:], prot[:, :])
                onesrow_f = pa.tile([1, 128], F32, tag="onesrow")
                nc.vector.memset(onesrow_f[:], 1.0)
                nc.vector.tensor_copy(ones_row_r[:], onesrow_f[:])
                # preload the exp ACT table while ACT is idle in phase A
                warm = pa.tile([1, 128], F32, tag="warm")
                nc.scalar.activation(warm[:], onesrow_f[:], EXP, scale=0.125)
                bq_sb = pa.tile([128, QT_TILES], F32, tag="bq")
                bk_sb = pa.tile([128, QT_TILES], F32, tag="bk")
                nc.sync.dma_start(bq_sb[:], bq[:, :])
                nc.sync.dma_start(bk_sb[:], bk[:, :])
                nc.sync.dma_start(bv_r[:], bv[:, :])

                # direct fp32r DMA loads: wq first (projection starts ASAP),
                # x per chunk, then wk/wv
                def load_w(wt):
                    w_r = pa_w.tile([128, KCH, DHL], F32R, tag="wr")
                    nc.sync.dma_start(
                        w_r[:], wt.rearrange("(k p) n -> p k n", p=128))
                    return w_r

                wq_r = load_w(wq)
                xT_r = pa.tile([128, KCH, S], F32R)
                for k in range(KCH):
                    nc.sync.dma_start(xT_r[:, k, :], xT[k * 128:(k + 1) * 128, :])
                wk_r = load_w(wk)
                wv_r = load_w(wv)

                # Q/K projections + RoPE, q-blocked
                for w_r, b_sb, dest in ((wq_r, bq_sb, qrope_r),
                                        (wk_r, bk_sb, krope_r)):
                    for mt in range(QT_TILES):
                        for qb_i in range(NQB):
                            q0 = qb_i * QB
                            ps = ps_a.tile([128, QB], F32, tag="qkps",
                                           bufs=2)
                            for nq in range(QB // 512):
                                for k in range(KCH):
                                    nc.tensor.matmul(
                                        ps[:, nq * 512:(nq + 1) * 512],
                                        w_r[:, k, mt * 128:(mt + 1) * 128],
                                        xT_r[:, k, q0 + nq * 512:q0 + (nq + 1) * 512],
                                        start=(k == 0), stop=(k == KCH - 1))
                            qb_r = pa_t.tile([128, QB], F32R, tag="qb")
                            nc.vector.tensor_scalar(
                                out=qb_r[:], in0=ps[:],
                                scalar1=b_sb[:, mt:mt + 1],
                                scalar2=None, op0=ADD)
                            ps2 = ps_a.tile([128, QB], F32, tag="rotps")
                            for nq in range(QB // 512):
                                nc.tensor.matmul(
                                    ps2[:, nq * 512:(nq + 1) * 512],
                                    prot_r[:, :],
                                    qb_r[:, nq * 512:(nq + 1) * 512],
                                    start=True, stop=True)
                            t1 = pa_t.tile([128, QB], F32, tag="t1")
                            nc.vector.scalar_tensor_tensor(
                                out=t1[:], in0=ps[:],
                                scalar=b_sb[:, mt:mt + 1],
                                in1=cos_sb[:, q0:q0 + QB], op0=ADD, op1=MULT)
                            t2 = pa_t.tile([128, QB], F32, tag="t2")
                            nc.vector.tensor_mul(t2[:], ps2[:],
                                                 sin_sb[:, q0:q0 + QB])
                            nc.vector.tensor_add(dest[:, mt, q0:q0 + QB],
                                                 t1[:], t2[:])

                # V projection (same psum scope -> can interleave)
                nc.vector.tensor_copy(
                    v_r[:, :, :, DH:DH + 1],
                    ones_col_f[:, None, None, :].broadcast_to([128, SKT, HL, 1]))
                for sk in range(SKT):
                    vps = ps_a.tile([128, DHL], F32, tag="vps", bufs=2)
                    for k in range(KCH):
                        nc.tensor.matmul(
                            vps[:], xT_r[:, k, sk * 128:(sk + 1) * 128],
                            wv_r[:, k, :],
                            start=(k == 0), stop=False)
                    nc.tensor.matmul(vps[:], ones_row_r[:], bv_r[:],
                                     start=False, stop=True)
                    nc.scalar.activation(
                        v_r[:, sk, :, 0:DH],
                        vps[:].rearrange("p (h d) -> p h d", h=HL), COPY)

            # ================= PHASE B (q-blocked) =================
            with (
                tc.tile_pool(name="pb", bufs=1) as pb,
                tc.tile_pool(name="pb_exp", bufs=exp_bufs) as pb_exp,
                tc.tile_pool(name="pb_n", bufs=2) as pb_n,
                tc.tile_pool(name="pc", bufs=1) as pc,
                tc.tile_pool(name="pc_y", bufs=y_bufs) as pc_y,
            ):
                outT_r = pb.tile([128, QT_TILES, S], F32R)
                wo_r = pc.tile([128, QT_TILES, DM], F32R)
                nc.sync.dma_start(wo_r[:], wo[:, :, :])

                ps_b_ctx = tc.tile_pool(name="ps_b", bufs=1, space="PSUM")
                ps_b = ps_b_ctx.__enter__()
                for qb_i in range(NQB):
                    q0 = qb_i * QB
                    for h in range(HL):
                        mt = h // 2
                        half = (h % 2) * DH
                        qt_h = qrope_r[half:half + DH, mt, :]
                        kt_h = krope_r[half:half + DH, mt, :]
                        pv_ps = ps_b.tile([DH + 1, QB], F32, tag="pvps",
                                          bufs=pv_bufs)
                        for sk in range(SKT):
                            st_ps = ps_b.tile([128, QB], F32, tag="stps",
                                              bufs=st_bufs)
                            for nq in range(QB // 512):
                                nc.tensor.matmul(
                                    st_ps[:, nq * 512:(nq + 1) * 512],
                                    kt_h[:, sk * 128:(sk + 1) * 128],
                                    qt_h[:, q0 + nq * 512:q0 + (nq + 1) * 512],
                                    start=True, stop=True)
                            expst = pb_exp.tile([128, QB], F32R, tag="expst")
                            nc.scalar.activation(expst[:], st_ps[:], EXP,
                                                 scale=0.125)
                            for nq in range(QB // 512):
                                nc.tensor.matmul(
                                    pv_ps[:, nq * 512:(nq + 1) * 512],
                                    v_r[:, sk, h, :],
                                    expst[:, nq * 512:(nq + 1) * 512],
                                    start=(sk == 0), stop=(sk == SKT - 1))
                        den_t = pb_n.tile([1, QB], F32, tag="dent")
                        nc.vector.tensor_copy(den_t[0:1, :],
                                              pv_ps[DH:DH + 1, :])
                        pvf = pb_n.tile([DH, QB], F32, tag="pvf")
                        nc.vector.tensor_copy(pvf[:], pv_ps[0:DH, :])
                        rec_b = pb_n.tile([DH, QB], F32, tag="recb")
                        nc.gpsimd.partition_broadcast(rec_b[:], den_t[0:1, :])
                        nc.vector.reciprocal_approx_fast(out=rec_b[:],
                                                         in_=rec_b[:])
                        nc.vector.tensor_mul(
                            outT_r[half:half + DH, mt, q0:q0 + QB],
                            pvf[:], rec_b[:])
                    # phase C for this q block (hidden under next block's B;
                    # the last block reuses the stps slots for pipelining)
                    last = qb_i == NQB - 1
                    for qt in range(q0 // 128, (q0 + QB) // 128):
                        y_ps = ps_b.tile([128, DM], F32,
                                         tag="stps" if last else "yps",
                                         bufs=st_bufs if last else 1)
                        for kc in range(QT_TILES):
                            for c2 in range(DM // 512):
                                nc.tensor.matmul(
                                    y_ps[:, c2 * 512:(c2 + 1) * 512],
                                    outT_r[:, kc, qt * 128:(qt + 1) * 128],
                                    wo_r[:, kc, c2 * 512:(c2 + 1) * 512],
                                    start=(kc == 0), stop=(kc == QT_TILES - 1))
                        y_sb = pc_y.tile([128, DM], F32, tag="ysb")
                        nc.vector.tensor_copy(y_sb[:], y_ps[:])
                        nc.sync.dma_start(y[qt * 128:(qt + 1) * 128, :], y_sb[:])
                ps_b_ctx.__exit__(None, None, None)

    nc.finalize()
    return nc



def _rope_tables():
    inv_freq = 1.0 / (10000.0 ** (np.arange(0, DH, 2, dtype=np.float32) / DH))
    ang = np.arange(S, dtype=np.float32)[:, None] * inv_freq[None, :]
    sin = np.concatenate([np.sin(ang), np.sin(ang)], axis=-1)  # [S, DH]
    cos = np.concatenate([np.cos(ang), np.cos(ang)], axis=-1)
    sinT = np.ascontiguousarray(np.vstack([sin.T, sin.T]), dtype=np.float32)
    cosT = np.ascontiguousarray(np.vstack([cos.T, cos.T]), dtype=np.float32)
    return sinT, cosT  # [128, S]


def _rot_matrix():
    half = DH // 2
    m64 = np.zeros((DH, DH), dtype=np.float32)
    for d in range(half):
        m64[d + half, d] = -1.0       # rot[d] = -q[d+half]
    for d in range(half, DH):
        m64[d - half, d] = 1.0        # rot[d] = q[d-half]
    m = np.zeros((128, 128), dtype=np.float32)
    m[0:DH, 0:DH] = m64
    m[DH:, DH:] = m64
    return m


def _make_runner(nc):
    """Build a cached jitted SPMD executor (mirrors the multi-core tail of
    concourse.bass2jax.run_bass_via_pjrt so repeat calls skip recompiles)."""
    import jax
    import numpy as _np
    from jax.sharding import Mesh, PartitionSpec
    from jax.experimental.shard_map import shard_map
    from concourse import bass2jax, mybir as _mybir

    bass2jax.install_neuronx_cc_hook()

    partition_name = (
        nc.partition_id_tensor.name if nc.partition_id_tensor else None)
    in_names, out_names, out_avals, zero_shapes = [], [], [], []
    for alloc in nc.m.functions[0].allocations:
        if not isinstance(alloc, _mybir.MemoryLocationSet):
            continue
        name = alloc.memorylocations[0].name
        if alloc.kind == "ExternalInput":
            if name != partition_name:
                in_names.append(name)
        elif alloc.kind == "ExternalOutput":
            out_names.append(name)
            shape = tuple(alloc.tensor_shape)
            dtype = _mybir.dt.np(alloc.dtype)
            out_avals.append(jax.core.ShapedArray(shape, dtype))
            zero_shapes.append((shape, dtype))
    n_params = len(in_names)
    all_names = in_names + out_names
    if partition_name is not None:
        all_names = all_names + [partition_name]

    def _body(*args):
        operands = list(args)
        if partition_name is not None:
            operands.append(bass2jax.partition_id_tensor())
        outs = bass2jax._bass_exec_p.bind(
            *operands,
            out_avals=tuple(out_avals),
            in_names=tuple(all_names),
            out_names=tuple(out_names),
            lowering_input_output_aliases=(),
            sim_require_finite=True,
            sim_require_nnan=True,
            nc=nc,
        )
        return tuple(outs)

    devices = jax.devices()[:NCORES]
    mesh = Mesh(_np.asarray(devices), ("core",))
    n_outs = len(out_names)
    sharded = jax.jit(
        shard_map(
            _body, mesh=mesh,
            in_specs=(PartitionSpec("core"),) * (n_params + n_outs),
            out_specs=(PartitionSpec("core"),) * n_outs,
            check_rep=False,
        ),
        donate_argnums=tuple(range(n_params, n_params + n_outs)),
        keep_unused=True,
    )

    def run(in_maps):
        concat_in = [
            _np.concatenate([_np.asarray(m[name]) for m in in_maps], axis=0)
            for name in in_names
        ]
        concat_zeros = [
            _np.zeros((NCORES * s[0], *s[1:]), dt) for (s, dt) in zero_shapes
        ]
        out_arrs = sharded(*concat_in, *concat_zeros)
        return [
            {
                name: _np.asarray(out_arrs[i]).reshape(
                    NCORES, *out_avals[i].shape)[c]
                for i, name in enumerate(out_names)
            }
            for c in range(NCORES)
        ]

    return run


def _get_runner():
    if "runner" not in _CACHE:
        nc = _build_nc()
        _CACHE["nc"] = nc
        _CACHE["runner"] = _make_runner(nc)
    return _CACHE["runner"]


def make_in_maps(x, wq, bq, wk, bk, wv, bv, wo, bo):
    """Build the 8 per-core input dicts from full inputs."""
    x = np.asarray(x, dtype=np.float32)
    if "tables" not in _CACHE:
        _CACHE["tables"] = _rope_tables()
        _CACHE["prot"] = _rot_matrix()
    sinT, cosT = _CACHE["tables"]
    prot = _CACHE["prot"]
    in_maps = []
    for c in range(NCORES):
        b, hg = divmod(c, HL)
        sl = slice(hg * DHL, (hg + 1) * DHL)
        in_maps.append({
            "xT": np.ascontiguousarray(x[b].T),
            "wq": np.ascontiguousarray(np.asarray(wq, np.float32)[:, sl]),
            "wk": np.ascontiguousarray(np.asarray(wk, np.float32)[:, sl]),
            "wv": np.ascontiguousarray(np.asarray(wv, np.float32)[:, sl]),
            "wo": np.ascontiguousarray(
                np.asarray(wo, np.float32)[sl, :].reshape(QT_TILES, 128, DM)
                .transpose(1, 0, 2)),
            "bq": np.ascontiguousarray(
                np.asarray(bq, np.float32)[sl].reshape(QT_TILES, 128).T),
            "bk": np.ascontiguousarray(
                np.asarray(bk, np.float32)[sl].reshape(QT_TILES, 128).T),
            "bv": np.ascontiguousarray(
                np.asarray(bv, np.float32)[sl].reshape(1, DHL)),
            "cosT": cosT,
            "sinT": sinT,
            "prot": prot,
        })
    return in_maps


def kernel(x, wq, bq, wk, bk, wv, bv, wo, bo):
    runner = _get_runner()
    in_maps = make_in_maps(x, wq, bq, wk, bk, wv, bv, wo, bo)
    results = runner(in_maps)
    bo = np.asarray(bo, dtype=np.float32)
    out = np.empty((B, S, DM), dtype=np.float32)
    for b in range(B):
        acc = results[b * HL + 0]["y"].astype(np.float32, copy=True)
        for hg in range(1, HL):
            acc += results[b * HL + hg]["y"]
        out[b] = acc + bo[None, :]
    return out



# revision 41
# speedup vs baseline: 1.1729x; 1.1729x over previous
"""MultiHeadAttention + RoPE kernel for 8 Trainium2 NeuronCores.

Sharding: core c in 0..7 -> batch b = c//4, head-group hg = c%4 (4 heads
each).  Each core computes its 4 heads' attention for its batch and a
partial output projection y_part = out_heads @ wo[head rows]; the host
sums the 4 partials per batch (bf16 -> f32) and adds bo.

Per-core dataflow:
  - x arrives in four 512-query column-block DMAs; Q/K projections
    (fp32r, transposed [depth, S]) and V (natural, bf16 + ones column)
    are paced into the attention pipeline by a debt-based filler
    scheduler so the ACT engine's exp stream starts early and PE
    rarely idles
  - RoPE: dest = (acc+b)*cos + blockswap((acc+b)*sin_signed), where the
    sign of sin is folded into the host table so the rotation is a pure
    permutation matmul
  - scores transposed: matmul(lhsT=KT tile, rhs=QT) -> ST [128 keys,
    2x512 queries]; exp on ACT (scale=1/8), output bf16
  - PV natural: exp'd scores are the STATIONARY operand (ldweights is
    free), V' [128,65] bf16 moving -> out_nat [128 q, 65] accumulated
    over 16 key tiles in PSUM; col 64 = softmax denominator.  The four
    per-bank accumulators share one PSUM zero-region, so a whole-bank
    zero-write matmul opens each accumulation epoch and the PV matmuls
    run with start=False/skip_group_check
  - normalize: DVE reciprocal + per-partition scale during PSUM evac
    (bf16); head pairs assembled to [128 q, 128 d] and PE-transposed
    to [128 d, q] for the O-projection (gpsimd cannot touch PSUM, so
    all PSUM-side elementwise work lives on DVE/ACT)
  - O-proj per query block: matmul(lhsT=outT pair tile, rhs=wo bf16),
    y stored bf16 via paired-row DMAs; host sums partials + bo in f32
"""

import numpy as np

import concourse.bacc as bacc
import concourse.mybir as mybir
from concourse.tile import TileContext

try:  # persistent XLA compile cache: repeat processes skip the ~4min compile
    import jax as _jax
    _jax.config.update("jax_compilation_cache_dir", "/tmp/jax_comp_cache")
    _jax.config.update("jax_persistent_cache_min_compile_time_secs", 1.0)
except Exception:
    pass

B, S, DM, H, DH = 2, 2048, 1024, 16, 64
NCORES = 8
HL = 4                 # heads per core
DHL = HL * DH          # 256
KCH = DM // 128        # 8 k-chunks of the model-dim contraction
SKT = S // 128         # 16 key tiles
QB = 1024              # phase-B query block
NQB = S // QB          # 2

F32 = mybir.dt.float32
F32R = mybir.dt.float32r
BF16 = mybir.dt.bfloat16
EXP = mybir.ActivationFunctionType.Exp
COPY = mybir.ActivationFunctionType.Copy
ADD = mybir.AluOpType.add
MULT = mybir.AluOpType.mult

_CACHE = {}


def _build_nc():
    nc = bacc.Bacc()
    xT = nc.dram_tensor("xT", [DM, S], F32R, kind="ExternalInput")
    wq = nc.dram_tensor("wq", [DM, DHL], F32R, kind="ExternalInput")
    wk = nc.dram_tensor("wk", [DM, DHL], F32R, kind="ExternalInput")
    wv = nc.dram_tensor("wv", [DM, DHL], F32R, kind="ExternalInput")
    wo = nc.dram_tensor("wo", [128, 2, DM], BF16, kind="ExternalInput")
    cosT = nc.dram_tensor("cosT", [128, S], F32, kind="ExternalInput")
    sinT = nc.dram_tensor("sinT", [128, S], F32, kind="ExternalInput")
    # constsA (f32r): prot [0:128], bv row0 [128:384]
    # constsB (f32): bq [0:2], bk [2:4], cos qb0 [4:516], sin qb0 [516:1028]
    constsA = nc.dram_tensor("constsA", [128, 384], F32R,
                             kind="ExternalInput")
    constsB = nc.dram_tensor("constsB", [128, 1028], F32,
                             kind="ExternalInput")
    ident = nc.dram_tensor("ident", [128, 128], BF16, kind="ExternalInput")
    y = nc.dram_tensor("y", [S, DM], BF16, kind="ExternalOutput")
    yre = y.rearrange("(a p) n -> p a n", p=128)

    with TileContext(nc) as tc:
        with tc.tile_pool(name="p0", bufs=1) as p0:
            # persistent SBUF
            qrope_r = p0.tile([128, 2, S], F32R)
            krope_r = p0.tile([128, 2, S], F32R)
            v_r = p0.tile([128, SKT, HL, DH + 1], BF16)
            outT_sb = p0.tile([128, 2, S], BF16)
            xT_r = p0.tile([128, KCH, S], F32R)
            wq_r = p0.tile([128, KCH, DHL], F32R)
            wk_r = p0.tile([128, KCH, DHL], F32R)
            wv_r = p0.tile([128, KCH, DHL], F32R)
            wo_r = p0.tile([128, 2, DM], BF16)
            cos_sb = p0.tile([128, S], F32)
            sin_sb = p0.tile([128, S], F32)
            cA = p0.tile([128, 384], F32R)
            cB = p0.tile([128, 1028], F32)
            ident_r = p0.tile([128, 128], BF16)
            ones_row_r = p0.tile([1, 128], F32R)
            zrow = p0.tile([1, 512], BF16)
            warm = p0.tile([1, 128], F32)
            prot_r = cA[:, 0:128]
            bv_r = cA[0:1, 128:128 + DHL]
            bq_sb = cB[:, 0:2]
            bk_sb = cB[:, 2:4]

            def load_w(dst, src):
                nc.sync.dma_start(
                    dst[:], src.rearrange("(k p) n -> p k n", p=128))

            def load_xqb(qb, cs_first=False):
                q0 = qb * 512

                def cs():
                    if qb > 0:
                        nc.sync.dma_start(cos_sb[:, q0:q0 + 512],
                                          cosT[:, q0:q0 + 512])
                        nc.sync.dma_start(sin_sb[:, q0:q0 + 512],
                                          sinT[:, q0:q0 + 512])
                src = xT[:, q0:q0 + 512].rearrange("(k p) n -> p k n", p=128)
                if cs_first:
                    cs()
                nc.sync.dma_start(xT_r[:, :, q0:q0 + 512], src)
                if not cs_first:
                    cs()

            nc.sync.dma_start(cB[:], constsB[:, :])
            nc.sync.dma_start(cA[:], constsA[:, :])
            load_w(wq_r, wq)
            load_xqb(0)
            load_w(wk_r, wk)
            nc.vector.memset(warm[:], 1.0)
            nc.vector.tensor_copy(ones_row_r[:], warm[:])
            nc.vector.memset(zrow[:], 0.0)
            nc.vector.memset(v_r[:, :, :, DH:DH + 1], 1.0)
            # preload the exp ACT table while ACT is idle
            nc.scalar.activation(warm[:], warm[:], EXP, scale=0.125)
            load_xqb(1, cs_first=True)
            load_w(wv_r, wv)
            load_xqb(2)
            nc.sync.dma_start(ident_r[:], ident[:, :])
            load_xqb(3)
            nc.sync.dma_start(wo_r[:], wo[:, :, :])

            with (
                tc.tile_pool(name="pb_exp", bufs=3) as pb_exp,
                tc.tile_pool(name="pb_sm", bufs=2) as pb_sm,
                tc.tile_pool(name="ps_b", bufs=2, space="PSUM") as ps_b,
            ):
                # ---------- emit helpers ----------
                def emit_proj_acc(qb, w_r, b_sb, dest, mt, ps_a, pa_t,
                                  eng=None):
                    """Projection accumulation + rope multiplies; returns a
                    closure emitting the rotation matmul + final add, to be
                    placed a couple of PE units later in the stream.  The
                    elementwise rope ops run on `eng` (DVE for Q, Pool for K
                    so the two chains run in parallel early on)."""
                    q0 = qb * 512
                    eng = eng or nc.vector
                    acc = ps_a.tile([128, 512], F32, tag="a", name="acc")
                    for c in range(KCH):
                        nc.tensor.matmul(
                            acc[:], w_r[:, c, mt * 128:(mt + 1) * 128],
                            xT_r[:, c, q0:q0 + 512],
                            start=(c == 0), stop=(c == KCH - 1))
                    cos_src = (cB[:, 4:516] if qb == 0
                               else cos_sb[:, q0:q0 + 512])
                    sin_src = (cB[:, 516:1028] if qb == 0
                               else sin_sb[:, q0:q0 + 512])
                    u = pa_t.tile([128, 512], F32R, tag="u")
                    eng.scalar_tensor_tensor(
                        out=u[:], in0=acc[:], scalar=b_sb[:, mt:mt + 1],
                        in1=sin_src, op0=ADD, op1=MULT)
                    t1 = pa_t.tile([128, 512], F32, tag="t1")
                    eng.scalar_tensor_tensor(
                        out=t1[:], in0=acc[:], scalar=b_sb[:, mt:mt + 1],
                        in1=cos_src, op0=ADD, op1=MULT)

                    def finish_rot():
                        rot = ps_a.tile([128, 512], F32, tag="a", name="rot")
                        nc.tensor.matmul(rot[:], prot_r[:, :], u[:],
                                         start=True, stop=True)
                        eng.tensor_add(dest[:, mt, q0:q0 + 512],
                                       t1[:], rot[:])
                    return finish_rot

                def emit_v(sk, ps_a):
                    vps = ps_a.tile([128, 512], F32, tag="a", name="vps")
                    for c in range(KCH):
                        nc.tensor.matmul(
                            vps[:, 0:DHL],
                            xT_r[:, c, sk * 128:(sk + 1) * 128],
                            wv_r[:, c, :], start=(c == 0), stop=False)
                    nc.tensor.matmul(vps[:, 0:DHL], ones_row_r[:], bv_r[:],
                                     start=False, stop=True)
                    nc.gpsimd.tensor_copy(
                        v_r[:, sk, :, 0:DH],
                        vps[:, 0:DHL].rearrange("p (h d) -> p h d", h=HL))

                def emit_st_exp(qi, h, sk, tag="expst", bufs=5,
                                split=False):
                    q0 = qi * QB
                    mt = h // 2
                    half = (h % 2) * DH
                    qt_h = qrope_r[half:half + DH, mt, :]
                    kt_h = krope_r[half:half + DH, mt, :]
                    st = ps_b.tile([128, 2, 512], F32, tag="st", name="st")
                    expst = pb_exp.tile([128, 2, 512], BF16, tag=tag,
                                        bufs=bufs, name="expst")
                    for n in range(2):
                        nc.tensor.matmul(
                            st[:, n, :],
                            kt_h[:, sk * 128:(sk + 1) * 128],
                            qt_h[:, q0 + n * 512:q0 + (n + 1) * 512],
                            start=True, stop=True)
                        if split:
                            nc.scalar.activation(expst[:, n, :], st[:, n, :],
                                                 EXP, scale=0.125)
                    if not split:
                        nc.scalar.activation(expst[:], st[:], EXP, scale=0.125)
                    return expst

                def emit_pv(h, accs, sk, expst):
                    # accumulators share PSUM banks, so groups are managed
                    # by the whole-bank clear matmul in new_accs
                    for qt in range(8):
                        nc.tensor.matmul(
                            accs[qt // 4][:, qt % 4, 0:DH + 1],
                            expst[:, qt // 4,
                                  (qt % 4) * 128:(qt % 4 + 1) * 128],
                            v_r[:, sk, h, :],
                            start=False, stop=False, skip_group_check=True)

                def emit_b_chunk(qi, h, accs, sk_lo, sk_hi):
                    for sk in range(sk_lo, sk_hi):
                        expst = emit_st_exp(qi, h, sk)
                        emit_pv(h, accs, sk, expst)

                def emit_h_finish(h, accs, onat):
                    half = (h % 2) * DH
                    rec = pb_sm.tile([128, 8, 1], F32, tag="rec")
                    for g in range(2):
                        nc.vector.reciprocal(rec[:, g * 4:(g + 1) * 4, :],
                                             accs[g][:, :, DH:DH + 1])
                        for qt in range(g * 4, g * 4 + 4):
                            eng = nc.vector if qt % 2 == 0 else nc.gpsimd
                            eng.tensor_scalar(
                                out=onat[:, qt, half:half + DH],
                                in0=accs[qt // 4][:, qt % 4, 0:DH],
                                scalar1=rec[:, qt, :],
                                scalar2=None, op0=MULT)

                def emit_pair(qi, p, onat):
                    q0 = qi * QB
                    for qt in range(8):
                        pt = ps_b.tile([128, 128], BF16, tag="out", name="pt")
                        nc.tensor.transpose(pt[:], onat[:, qt, :], ident_r[:])
                        nc.vector.tensor_copy(
                            outT_sb[:, p, q0 + qt * 128:q0 + (qt + 1) * 128],
                            pt[:])

                def emit_oproj(qi, pair_i, ps_y, pc_y, tail=False):
                    # one pair of query tiles -> one y DMA
                    ysb = pc_y.tile([128, 2, DM], BF16, tag="ysb", bufs=3)
                    for j in range(2):
                        qt = qi * 8 + pair_i * 2 + j
                        for oc in range(2):
                            yp = ps_y.tile([128, 512], F32, tag="y")
                            for p in range(2):
                                nc.tensor.matmul(
                                    yp[:],
                                    outT_sb[:, p, qt * 128:(qt + 1) * 128],
                                    wo_r[:, p, oc * 512:(oc + 1) * 512],
                                    start=(p == 0), stop=(p == 1))
                            if tail and oc == 0:
                                # ACT is idle at the tail; use it for evac
                                nc.scalar.activation(
                                    ysb[:, j, oc * 512:(oc + 1) * 512],
                                    yp[:], COPY)
                            else:
                                eng = nc.vector if oc == 0 else nc.gpsimd
                                eng.tensor_copy(
                                    ysb[:, j, oc * 512:(oc + 1) * 512], yp[:])
                    qp = qi * 4 + pair_i
                    nc.sync.dma_start(yre[:, 2 * qp:2 * qp + 2, :], ysb[:])

                def new_accs(tag_i):
                    accs = [ps_b.tile([128, 4, 128], F32, tag="out",
                                      name=f"acc{tag_i}{g}") for g in range(2)]
                    for a in accs:
                        # whole-bank zero-write opens the accumulation epoch
                        # and orders (WAW) ahead of every PV matmul
                        nc.tensor.matmul(a[:, :, :], zrow[0:1, 0:128],
                                         zrow[:], start=True, stop=True)
                    return accs

                # ---------- phase A interleaved with head 0 of qB0 ----------
                with (
                    tc.tile_pool(name="pa_t", bufs=3) as pa_t,
                    tc.tile_pool(name="ps_a", bufs=2, space="PSUM") as ps_a,
                ):
                    pending = []

                    def P(qb, w, mt):
                        wr, bs, dst, eng = ((wq_r, bq_sb, qrope_r, nc.vector)
                                            if w == "q" else
                                            (wk_r, bk_sb, krope_r, nc.gpsimd))
                        pending.append(
                            emit_proj_acc(qb, wr, bs, dst, mt, ps_a, pa_t,
                                          eng))

                    def Rc():
                        pending.pop(0)()

                    # --- filler schedule: phase A work paced into the gaps
                    # of the attention pipeline (PE is the global bottleneck;
                    # ACT-bound stretches leave ~0.35us/exp of PE spare) ---
                    accs0 = new_accs(0)
                    onat0 = pb_sm.tile([128, 8, 128], BF16, tag="onat",
                                       name="onat0")
                    dpv = {}     # sk -> deferred (h, accs, expst) for V units

                    def V(sk):
                        def go():
                            emit_v(sk, ps_a)
                            if sk in dpv:
                                h, accs, e = dpv.pop(sk)
                                emit_pv(h, accs, sk, e)
                        return go

                    fillers = []
                    costs = []

                    def F(fn, cost):
                        fillers.append(fn)
                        costs.append(cost)

                    PC, RC, VC = 1.7, 0.25, 1.0
                    F(lambda: P(0, "q", 0), PC)      # 0
                    F(lambda: P(0, "k", 0), PC)      # 1
                    F(Rc, RC)                        # 2
                    F(lambda: P(1, "q", 0), PC)      # 3
                    F(Rc, RC)                        # 4
                    F(Rc, RC)                        # 5  mt0 ropes qb0/1 done
                    F(lambda: P(1, "k", 0), PC)      # 6
                    F(Rc, RC)                        # 7  k10
                    for sk in range(8):              # 8-15: V0-7 (+ deferred PVs)
                        F(V(sk), VC)
                    F(lambda: P(2, "k", 0), PC)      # 16
                    F(Rc, RC)                        # 17 k20
                    for sk in range(8, 12):          # 18-21
                        F(V(sk), VC)
                    F(lambda: P(3, "k", 0), PC)      # 22
                    F(Rc, RC)                        # 23 k30
                    for sk in range(12, 16):         # 24-27
                        F(V(sk), VC)
                    F(lambda: P(0, "q", 1), PC)      # 28
                    F(lambda: P(0, "k", 1), PC)      # 29
                    F(Rc, RC)                        # 30
                    F(Rc, RC)                        # 31 mt1 ropes qb0
                    F(lambda: P(1, "q", 1), PC)      # 32
                    F(lambda: P(1, "k", 1), PC)      # 33
                    F(Rc, RC)                        # 34
                    F(Rc, RC)                        # 35 mt1 ropes qb1
                    F(lambda: P(2, "q", 1), PC)      # 36
                    F(lambda: P(2, "k", 1), PC)      # 37
                    F(Rc, RC)                        # 38
                    F(Rc, RC)                        # 39
                    F(lambda: P(3, "q", 1), PC)      # 40
                    F(lambda: P(3, "k", 1), PC)      # 41
                    F(Rc, RC)                        # 42
                    F(Rc, RC)                        # 43 mt1 ropes qb2/3
                    F(lambda: P(2, "q", 0), PC)      # 44
                    F(Rc, RC)                        # 45
                    F(lambda: P(3, "q", 0), PC)      # 46
                    F(Rc, RC)                        # 47 q-mt0 qb2/3 (for qB1)

                    state = {"next": 0, "debt": 0.0}

                    def pop_to(n):
                        while state["next"] <= n:
                            fillers[state["next"]]()
                            state["debt"] -= costs[state["next"]]
                            state["next"] += 1

                    def spare(amt):
                        state["debt"] += amt
                        while (state["next"] < len(fillers)
                               and costs[state["next"]] <= state["debt"]):
                            fillers[state["next"]]()
                            state["debt"] -= costs[state["next"]]
                            state["next"] += 1

                    # prewarm the PE p-state during the initial DMA wait
                    warm_ps = ps_a.tile([128, 512], F32, tag="a",
                                        name="warmps")
                    ones_b = ones_row_r[0:1, 0:1].broadcast_to([1, 512])
                    for _ in range(10):
                        nc.tensor.matmul(warm_ps[:], ones_row_r[:], ones_b,
                                         start=True, stop=True)

                    # h0: first 8 score tiles exp'd with deferred PVs (the
                    # V projections haven't run yet)
                    pop_to(5)
                    for sk in range(4):
                        dpv[sk] = (0, accs0,
                                   emit_st_exp(0, 0, sk, tag="expst1",
                                               bufs=8, split=True))
                    pop_to(7)
                    for sk in range(4, 8):
                        dpv[sk] = (0, accs0,
                                   emit_st_exp(0, 0, sk, tag="expst1",
                                               bufs=8, split=True))
                        spare(0.45)
                    # h1's first 8 likewise (slots free as V units run)
                    h1_saved = []
                    for sk in range(8):
                        pop_to(8 + sk)
                        h1_saved.append(
                            emit_st_exp(0, 1, sk, tag="expst1", bufs=8))
                        spare(0.45)
                    # h0 second half: full chunks
                    for sk in range(8, 16):
                        pop_to(17 if sk < 12 else 23)
                        pop_to((18 + sk - 8) if sk < 12 else (24 + sk - 12))
                        emit_b_chunk(0, 0, accs0, sk, sk + 1)
                        spare(0.45)
                    emit_h_finish(0, accs0, onat0)
                    # h1: deferred PVs + remaining chunks
                    accs1 = new_accs(1)
                    for sk in range(8):
                        emit_pv(1, accs1, sk, h1_saved[sk])
                    for sk in range(8, 16):
                        emit_b_chunk(0, 1, accs1, sk, sk + 1)
                        spare(0.45)
                    pop_to(35)  # mt1 qb0/1 ropes must precede head 2 scores
                    carry02 = [emit_st_exp(0, 2, k, bufs=5) for k in range(3)]
                    emit_h_finish(1, accs1, onat0)
                    emit_pair(0, 0, onat0)
                    # qB0 heads 2-3 (need the mt1 fillers)
                    onat01 = pb_sm.tile([128, 8, 128], BF16, tag="onat",
                                        name="onat01")
                    carry = carry02
                    for h in (2, 3):
                        accs = new_accs(h)
                        for sk, e in enumerate(carry):
                            emit_pv(h, accs, sk, e)
                        for sk in range(len(carry), SKT):
                            pop_to(31 if sk < 4 else
                                   (35 if sk < 8 else
                                    (39 if sk < 12 else 43)))
                            emit_b_chunk(0, h, accs, sk, sk + 1)
                            spare(0.45)
                        if h == 2:
                            carry = [emit_st_exp(0, 3, k, bufs=5) for k in range(3)]
                        else:
                            pop_to(47)
                            carry = [emit_st_exp(1, 0, k, bufs=5) for k in range(3)]
                        emit_h_finish(h, accs, onat01)
                    emit_pair(0, 1, onat01)

                # ---------- qB1 + per-block O-proj ----------
                with (
                    tc.tile_pool(name="pc_y", bufs=2) as pc_y,
                    tc.tile_pool(name="ps_y", bufs=2, space="PSUM") as ps_y,
                ):
                    seq = [(1, 0), (1, 1), (1, 2), (1, 3)]
                    onats = {}
                    for idx, (qi, h) in enumerate(seq):
                        pair = (qi, h // 2)
                        if pair not in onats:
                            onats[pair] = pb_sm.tile(
                                [128, 8, 128], BF16, tag="onat",
                                name=f"onat{qi}{h//2}")
                        accs = new_accs(f"{qi}{h}")
                        for sk, e in enumerate(carry):
                            emit_pv(h, accs, sk, e)
                        emit_b_chunk(qi, h, accs, len(carry), SKT)
                        # pre-emit the next head's first scores so ACT has
                        # work across the head boundary
                        carry = []
                        if idx + 1 < len(seq):
                            nqi, nh = seq[idx + 1]
                            carry = [emit_st_exp(nqi, nh, k, bufs=5) for k in range(3)]
                        emit_h_finish(h, accs, onats[pair])
                        if h % 2 == 1:
                            emit_pair(qi, h // 2, onats[pair])
                        emit_oproj(0, h, ps_y, pc_y)
                    # tail: qB1's O-proj
                    for pair_i in range(4):
                        emit_oproj(1, pair_i, ps_y, pc_y, tail=True)

    nc.finalize()
    return nc


def _rope_tables():
    inv_freq = 1.0 / (10000.0 ** (np.arange(0, DH, 2, dtype=np.float32) / DH))
    ang = np.arange(S, dtype=np.float32)[:, None] * inv_freq[None, :]
    sin = np.concatenate([np.sin(ang), np.sin(ang)], axis=-1)  # [S, DH]
    cos = np.concatenate([np.cos(ang), np.cos(ang)], axis=-1)
    # fold the rotate-half signs into sin: rows d%64 >= 32 are negated,
    # so the rotation becomes a pure block-swap permutation
    ssin = sin.copy()
    ssin[:, DH // 2:] = -ssin[:, DH // 2:]
    sinT = np.ascontiguousarray(np.vstack([ssin.T, ssin.T]), dtype=np.float32)
    cosT = np.ascontiguousarray(np.vstack([cos.T, cos.T]), dtype=np.float32)
    return sinT, cosT  # [128, S]


def _rot_matrix():
    # pure block-swap: out[d] = u[d+32] (d%64 < 32), u[d-32] (d%64 >= 32)
    half = DH // 2
    m64 = np.zeros((DH, DH), dtype=np.float32)
    for d in range(half):
        m64[d + half, d] = 1.0
        m64[d, d + half] = 1.0
    m = np.zeros((128, 128), dtype=np.float32)
    m[0:DH, 0:DH] = m64
    m[DH:, DH:] = m64
    return m


def _make_runner(nc):
    """Build a cached jitted SPMD executor (mirrors the multi-core tail of
    concourse.bass2jax.run_bass_via_pjrt so repeat calls skip recompiles)."""
    import jax
    import numpy as _np
    from jax.sharding import Mesh, PartitionSpec
    from jax.experimental.shard_map import shard_map
    from concourse import bass2jax, mybir as _mybir

    bass2jax.install_neuronx_cc_hook()

    partition_name = (
        nc.partition_id_tensor.name if nc.partition_id_tensor else None)
    in_names, out_names, out_avals, zero_shapes = [], [], [], []
    for alloc in nc.m.functions[0].allocations:
        if not isinstance(alloc, _mybir.MemoryLocationSet):
            continue
        name = alloc.memorylocations[0].name
        if alloc.kind == "ExternalInput":
            if name != partition_name:
                in_names.append(name)
        elif alloc.kind == "ExternalOutput":
            out_names.append(name)
            shape = tuple(alloc.tensor_shape)
            dtype = _mybir.dt.np(alloc.dtype)
            out_avals.append(jax.core.ShapedArray(shape, dtype))
            zero_shapes.append((shape, dtype))
    n_params = len(in_names)
    all_names = in_names + out_names
    if partition_name is not None:
        all_names = all_names + [partition_name]

    def _body(*args):
        operands = list(args)
        if partition_name is not None:
            operands.append(bass2jax.partition_id_tensor())
        outs = bass2jax._bass_exec_p.bind(
            *operands,
            out_avals=tuple(out_avals),
            in_names=tuple(all_names),
            out_names=tuple(out_names),
            lowering_input_output_aliases=(),
            sim_require_finite=True,
            sim_require_nnan=True,
            nc=nc,
        )
        return tuple(outs)

    devices = jax.devices()[:NCORES]
    mesh = Mesh(_np.asarray(devices), ("core",))
    n_outs = len(out_names)
    sharded = jax.jit(
        shard_map(
            _body, mesh=mesh,
            in_specs=(PartitionSpec("core"),) * (n_params + n_outs),
            out_specs=(PartitionSpec("core"),) * n_outs,
            check_rep=False,
        ),
        donate_argnums=tuple(range(n_params, n_params + n_outs)),
        keep_unused=True,
    )

    def run(in_maps):
        concat_in = [
            _np.concatenate([_np.asarray(m[name]) for m in in_maps], axis=0)
            for name in in_names
        ]
        concat_zeros = [
            _np.zeros((NCORES * s[0], *s[1:]), dt) for (s, dt) in zero_shapes
        ]
        out_arrs = sharded(*concat_in, *concat_zeros)
        return [
            {
                name: _np.asarray(out_arrs[i]).reshape(
                    NCORES, *out_avals[i].shape)[c]
                for i, name in enumerate(out_names)
            }
            for c in range(NCORES)
        ]

    return run


def _get_runner():
    if "runner" not in _CACHE:
        nc = _build_nc()
        _CACHE["nc"] = nc
        _CACHE["runner"] = _make_runner(nc)
    return _CACHE["runner"]


def make_in_maps(x, wq, bq, wk, bk, wv, bv, wo, bo):
    """Build the 8 per-core input dicts from full inputs."""
    import ml_dtypes
    bf16 = ml_dtypes.bfloat16
    x = np.asarray(x, dtype=np.float32)
    if "tables" not in _CACHE:
        _CACHE["tables"] = _rope_tables()
        _CACHE["prot"] = _rot_matrix()
        _CACHE["ident"] = np.eye(128, dtype=bf16)
    sinT, cosT = _CACHE["tables"]
    prot = _CACHE["prot"]
    ident = _CACHE["ident"]
    in_maps = []
    for c in range(NCORES):
        b, hg = divmod(c, HL)
        sl = slice(hg * DHL, (hg + 1) * DHL)
        constsA = np.zeros((128, 384), np.float32)
        constsA[:, 0:128] = prot
        constsA[0, 128:128 + DHL] = np.asarray(bv, np.float32)[sl]
        constsB = np.zeros((128, 1028), np.float32)
        constsB[:, 0:2] = np.asarray(bq, np.float32)[sl].reshape(2, 128).T
        constsB[:, 2:4] = np.asarray(bk, np.float32)[sl].reshape(2, 128).T
        constsB[:, 4:516] = cosT[:, 0:512]
        constsB[:, 516:1028] = sinT[:, 0:512]
        in_maps.append({
            "xT": np.ascontiguousarray(x[b].T),
            "wq": np.ascontiguousarray(np.asarray(wq, np.float32)[:, sl]),
            "wk": np.ascontiguousarray(np.asarray(wk, np.float32)[:, sl]),
            "wv": np.ascontiguousarray(np.asarray(wv, np.float32)[:, sl]),
            "wo": np.ascontiguousarray(
                np.asarray(wo, np.float32)[sl, :].reshape(2, 128, DM)
                .transpose(1, 0, 2)).astype(bf16),
            "constsA": constsA,
            "constsB": constsB,
            "cosT": cosT,
            "sinT": sinT,
            "ident": ident,
        })
    return in_maps


def kernel(x, wq, bq, wk, bk, wv, bv, wo, bo):
    runner = _get_runner()
    in_maps = make_in_maps(x, wq, bq, wk, bk, wv, bv, wo, bo)
    results = runner(in_maps)
    bo = np.asarray(bo, dtype=np.float32)
    out = np.empty((B, S, DM), dtype=np.float32)
    for b in range(B):
        acc = results[b * HL + 0]["y"].astype(np.float32)
        for hg in range(1, HL):
            acc += results[b * HL + hg]["y"].astype(np.float32)
        out[b] = acc + bo[None, :]
    return out


# revision 42
# speedup vs baseline: 1.1792x; 1.0054x over previous
"""MultiHeadAttention + RoPE kernel for 8 Trainium2 NeuronCores.

Sharding: core c in 0..7 -> batch b = c//4, head-group hg = c%4 (4 heads
each).  Each core computes its 4 heads' attention for its batch and a
partial output projection y_part = out_heads @ wo[head rows]; the host
sums the 4 partials per batch (bf16 -> f32) and adds bo.

Per-core dataflow:
  - x arrives in four 512-query column-block DMAs; Q/K projections
    (fp32r, transposed [depth, S]) and V (natural, bf16 + ones column)
    are paced into the attention pipeline by a debt-based filler
    scheduler so the ACT engine's exp stream starts early and PE
    rarely idles
  - RoPE: dest = (acc+b)*cos + blockswap((acc+b)*sin_signed), where the
    sign of sin is folded into the host table so the rotation is a pure
    permutation matmul
  - scores transposed: matmul(lhsT=KT tile, rhs=QT) -> ST [128 keys,
    2x512 queries]; exp on ACT (scale=1/8), output bf16
  - PV natural: exp'd scores are the STATIONARY operand (ldweights is
    free), V' [128,65] bf16 moving -> out_nat [128 q, 65] accumulated
    over 16 key tiles in PSUM; col 64 = softmax denominator.  The four
    per-bank accumulators share one PSUM zero-region, so a whole-bank
    zero-write matmul opens each accumulation epoch and the PV matmuls
    run with start=False/skip_group_check
  - normalize: DVE reciprocal + per-partition scale during PSUM evac
    (bf16); head pairs assembled to [128 q, 128 d] and PE-transposed
    to [128 d, q] for the O-projection (gpsimd cannot touch PSUM, so
    all PSUM-side elementwise work lives on DVE/ACT)
  - O-proj per query block: matmul(lhsT=outT pair tile, rhs=wo bf16),
    y stored bf16 via paired-row DMAs; host sums partials + bo in f32
"""

import numpy as np

import concourse.bacc as bacc
import concourse.mybir as mybir
from concourse.tile import TileContext

try:  # persistent XLA compile cache: repeat processes skip the ~4min compile
    import jax as _jax
    _jax.config.update("jax_compilation_cache_dir", "/tmp/jax_comp_cache")
    _jax.config.update("jax_persistent_cache_min_compile_time_secs", 1.0)
except Exception:
    pass

B, S, DM, H, DH = 2, 2048, 1024, 16, 64
NCORES = 8
HL = 4                 # heads per core
DHL = HL * DH          # 256
KCH = DM // 128        # 8 k-chunks of the model-dim contraction
SKT = S // 128         # 16 key tiles
QB = 1024              # phase-B query block
NQB = S // QB          # 2

F32 = mybir.dt.float32
F32R = mybir.dt.float32r
BF16 = mybir.dt.bfloat16
EXP = mybir.ActivationFunctionType.Exp
COPY = mybir.ActivationFunctionType.Copy
ADD = mybir.AluOpType.add
MULT = mybir.AluOpType.mult

_CACHE = {}


def _build_nc():
    nc = bacc.Bacc()
    xT = nc.dram_tensor("xT", [DM, S], F32R, kind="ExternalInput")
    wq = nc.dram_tensor("wq", [DM, DHL], F32R, kind="ExternalInput")
    wk = nc.dram_tensor("wk", [DM, DHL], F32R, kind="ExternalInput")
    wv = nc.dram_tensor("wv", [DM, DHL], F32R, kind="ExternalInput")
    wo = nc.dram_tensor("wo", [128, 2, DM], BF16, kind="ExternalInput")
    cosT = nc.dram_tensor("cosT", [128, S], F32, kind="ExternalInput")
    sinT = nc.dram_tensor("sinT", [128, S], F32, kind="ExternalInput")
    # constsA (f32r): prot [0:128], bv row0 [128:384]
    # constsB (f32): bq [0:2], bk [2:4], cos qb0 [4:516], sin qb0 [516:1028]
    constsA = nc.dram_tensor("constsA", [128, 384], F32R,
                             kind="ExternalInput")
    constsB = nc.dram_tensor("constsB", [128, 1028], F32,
                             kind="ExternalInput")
    ident = nc.dram_tensor("ident", [128, 128], BF16, kind="ExternalInput")
    y = nc.dram_tensor("y", [S, DM], BF16, kind="ExternalOutput")
    yre = y.rearrange("(a p) n -> p a n", p=128)

    with TileContext(nc) as tc:
        with tc.tile_pool(name="p0", bufs=1) as p0:
            # persistent SBUF
            qrope_r = p0.tile([128, 2, S], F32R)
            krope_r = p0.tile([128, 2, S], F32R)
            v_r = p0.tile([128, SKT, HL, DH + 1], BF16)
            outT_sb = p0.tile([128, 2, S], BF16)
            xT_r = p0.tile([128, KCH, S], F32R)
            wq_r = p0.tile([128, KCH, DHL], F32R)
            wk_r = p0.tile([128, KCH, DHL], F32R)
            wv_r = p0.tile([128, KCH, DHL], F32R)
            wo_r = p0.tile([128, 2, DM], BF16)
            cos_sb = p0.tile([128, S], F32)
            sin_sb = p0.tile([128, S], F32)
            cA = p0.tile([128, 384], F32R)
            cB = p0.tile([128, 1028], F32)
            ident_r = p0.tile([128, 128], BF16)
            ones_row_r = p0.tile([1, 128], F32R)
            zrow = p0.tile([1, 512], BF16)
            warm = p0.tile([1, 128], F32)
            prot_r = cA[:, 0:128]
            bv_r = cA[0:1, 128:128 + DHL]
            bq_sb = cB[:, 0:2]
            bk_sb = cB[:, 2:4]

            def load_w(dst, src):
                nc.sync.dma_start(
                    dst[:], src.rearrange("(k p) n -> p k n", p=128))

            def load_xqb(qb, cs_first=False):
                q0 = qb * 512

                def cs():
                    if qb > 0:
                        nc.sync.dma_start(cos_sb[:, q0:q0 + 512],
                                          cosT[:, q0:q0 + 512])
                        nc.sync.dma_start(sin_sb[:, q0:q0 + 512],
                                          sinT[:, q0:q0 + 512])
                src = xT[:, q0:q0 + 512].rearrange("(k p) n -> p k n", p=128)
                if cs_first:
                    cs()
                nc.sync.dma_start(xT_r[:, :, q0:q0 + 512], src)
                if not cs_first:
                    cs()

            nc.sync.dma_start(cB[:], constsB[:, :])
            nc.sync.dma_start(cA[:], constsA[:, :])
            load_w(wq_r, wq)
            load_xqb(0)
            load_w(wk_r, wk)
            nc.vector.memset(warm[:], 1.0)
            nc.vector.tensor_copy(ones_row_r[:], warm[:])
            nc.vector.memset(zrow[:], 0.0)
            nc.vector.memset(v_r[:, :, :, DH:DH + 1], 1.0)
            # preload the exp ACT table while ACT is idle
            nc.scalar.activation(warm[:], warm[:], EXP, scale=0.125)
            load_xqb(1, cs_first=True)
            load_w(wv_r, wv)
            load_xqb(2)
            nc.sync.dma_start(ident_r[:], ident[:, :])
            load_xqb(3)
            nc.sync.dma_start(wo_r[:], wo[:, :, :])

            with (
                tc.tile_pool(name="pb_exp", bufs=3) as pb_exp,
                tc.tile_pool(name="pb_sm", bufs=2) as pb_sm,
                tc.tile_pool(name="ps_b", bufs=2, space="PSUM") as ps_b,
            ):
                # ---------- emit helpers ----------
                def emit_proj_acc(qb, w_r, b_sb, dest, mt, ps_a, pa_t,
                                  eng=None):
                    """Projection accumulation + rope multiplies; returns a
                    closure emitting the rotation matmul + final add, to be
                    placed a couple of PE units later in the stream.  The
                    elementwise rope ops run on `eng` (DVE for Q, Pool for K
                    so the two chains run in parallel early on)."""
                    q0 = qb * 512
                    eng = eng or nc.vector
                    acc = ps_a.tile([128, 512], F32, tag="a", name="acc")
                    for c in range(KCH):
                        nc.tensor.matmul(
                            acc[:], w_r[:, c, mt * 128:(mt + 1) * 128],
                            xT_r[:, c, q0:q0 + 512],
                            start=(c == 0), stop=(c == KCH - 1))
                    cos_src = (cB[:, 4:516] if qb == 0
                               else cos_sb[:, q0:q0 + 512])
                    sin_src = (cB[:, 516:1028] if qb == 0
                               else sin_sb[:, q0:q0 + 512])
                    u = pa_t.tile([128, 512], F32R, tag="u")
                    eng.scalar_tensor_tensor(
                        out=u[:], in0=acc[:], scalar=b_sb[:, mt:mt + 1],
                        in1=sin_src, op0=ADD, op1=MULT)
                    t1 = pa_t.tile([128, 512], F32, tag="t1")
                    eng.scalar_tensor_tensor(
                        out=t1[:], in0=acc[:], scalar=b_sb[:, mt:mt + 1],
                        in1=cos_src, op0=ADD, op1=MULT)

                    def finish_rot():
                        rot = ps_a.tile([128, 512], F32, tag="a", name="rot")
                        nc.tensor.matmul(rot[:], prot_r[:, :], u[:],
                                         start=True, stop=True)
                        eng.tensor_add(dest[:, mt, q0:q0 + 512],
                                       t1[:], rot[:])
                    return finish_rot

                def emit_v(sk, ps_a):
                    vps = ps_a.tile([128, 512], F32, tag="a", name="vps")
                    for c in range(KCH):
                        nc.tensor.matmul(
                            vps[:, 0:DHL],
                            xT_r[:, c, sk * 128:(sk + 1) * 128],
                            wv_r[:, c, :], start=(c == 0), stop=False)
                    nc.tensor.matmul(vps[:, 0:DHL], ones_row_r[:], bv_r[:],
                                     start=False, stop=True)
                    nc.gpsimd.tensor_copy(
                        v_r[:, sk, :, 0:DH],
                        vps[:, 0:DHL].rearrange("p (h d) -> p h d", h=HL))

                def emit_st_exp(qi, h, sk, tag="expst", bufs=5,
                                split=False):
                    q0 = qi * QB
                    mt = h // 2
                    half = (h % 2) * DH
                    qt_h = qrope_r[half:half + DH, mt, :]
                    kt_h = krope_r[half:half + DH, mt, :]
                    st = ps_b.tile([128, 2, 512], F32, tag="st", name="st")
                    expst = pb_exp.tile([128, 2, 512], BF16, tag=tag,
                                        bufs=bufs, name="expst")
                    for n in range(2):
                        nc.tensor.matmul(
                            st[:, n, :],
                            kt_h[:, sk * 128:(sk + 1) * 128],
                            qt_h[:, q0 + n * 512:q0 + (n + 1) * 512],
                            start=True, stop=True)
                        if split:
                            nc.scalar.activation(expst[:, n, :], st[:, n, :],
                                                 EXP, scale=0.125)
                    if not split:
                        nc.scalar.activation(expst[:], st[:], EXP, scale=0.125)
                    return expst

                def emit_pv(h, accs, sk, expst):
                    # accumulators share PSUM banks, so groups are managed
                    # by the whole-bank clear matmul in new_accs
                    for qt in range(8):
                        nc.tensor.matmul(
                            accs[qt // 4][:, qt % 4, 0:DH + 1],
                            expst[:, qt // 4,
                                  (qt % 4) * 128:(qt % 4 + 1) * 128],
                            v_r[:, sk, h, :],
                            start=False, stop=False, skip_group_check=True)

                def emit_b_chunk(qi, h, accs, sk_lo, sk_hi):
                    for sk in range(sk_lo, sk_hi):
                        expst = emit_st_exp(qi, h, sk)
                        emit_pv(h, accs, sk, expst)

                def emit_h_finish(h, accs, onat):
                    half = (h % 2) * DH
                    rec = pb_sm.tile([128, 8, 1], F32, tag="rec")
                    for g in range(2):
                        nc.vector.reciprocal(rec[:, g * 4:(g + 1) * 4, :],
                                             accs[g][:, :, DH:DH + 1])
                        for qt in range(g * 4, g * 4 + 4):
                            eng = nc.vector if qt % 2 == 0 else nc.gpsimd
                            eng.tensor_scalar(
                                out=onat[:, qt, half:half + DH],
                                in0=accs[qt // 4][:, qt % 4, 0:DH],
                                scalar1=rec[:, qt, :],
                                scalar2=None, op0=MULT)

                def emit_pair(qi, p, onat):
                    q0 = qi * QB
                    for qt in range(8):
                        pt = ps_b.tile([128, 128], BF16, tag="out", name="pt")
                        nc.tensor.transpose(pt[:], onat[:, qt, :], ident_r[:])
                        nc.vector.tensor_copy(
                            outT_sb[:, p, q0 + qt * 128:q0 + (qt + 1) * 128],
                            pt[:])

                def emit_oproj(qi, pair_i, ps_y, pc_y, tail=False):
                    # one pair of query tiles -> one y DMA
                    ysb = pc_y.tile([128, 2, DM], BF16, tag="ysb", bufs=3)
                    for j in range(2):
                        qt = qi * 8 + pair_i * 2 + j
                        for oc in range(2):
                            yp = ps_y.tile([128, 512], F32, tag="y")
                            for p in range(2):
                                nc.tensor.matmul(
                                    yp[:],
                                    outT_sb[:, p, qt * 128:(qt + 1) * 128],
                                    wo_r[:, p, oc * 512:(oc + 1) * 512],
                                    start=(p == 0), stop=(p == 1))
                            if tail and oc == 0:
                                # ACT is idle at the tail; use it for evac
                                nc.scalar.activation(
                                    ysb[:, j, oc * 512:(oc + 1) * 512],
                                    yp[:], COPY)
                            else:
                                eng = nc.vector if oc == 0 else nc.gpsimd
                                eng.tensor_copy(
                                    ysb[:, j, oc * 512:(oc + 1) * 512], yp[:])
                    qp = qi * 4 + pair_i
                    nc.sync.dma_start(yre[:, 2 * qp:2 * qp + 2, :], ysb[:])

                def new_accs(tag_i):
                    accs = [ps_b.tile([128, 4, 128], F32, tag="out",
                                      name=f"acc{tag_i}{g}") for g in range(2)]
                    for a in accs:
                        # whole-bank zero-write opens the accumulation epoch
                        # and orders (WAW) ahead of every PV matmul
                        nc.tensor.matmul(a[:, :, :], zrow[0:1, 0:128],
                                         zrow[:], start=True, stop=True)
                    return accs

                # ---------- phase A interleaved with head 0 of qB0 ----------
                with (
                    tc.tile_pool(name="pa_t", bufs=3) as pa_t,
                    tc.tile_pool(name="ps_a", bufs=2, space="PSUM") as ps_a,
                ):
                    pending = []

                    def P(qb, w, mt):
                        wr, bs, dst, eng = ((wq_r, bq_sb, qrope_r, nc.vector)
                                            if w == "q" else
                                            (wk_r, bk_sb, krope_r, nc.gpsimd))
                        pending.append(
                            emit_proj_acc(qb, wr, bs, dst, mt, ps_a, pa_t,
                                          eng))

                    def Rc():
                        pending.pop(0)()

                    # --- filler schedule: phase A work paced into the gaps
                    # of the attention pipeline (PE is the global bottleneck;
                    # ACT-bound stretches leave ~0.35us/exp of PE spare) ---
                    accs0 = new_accs(0)
                    onat0 = pb_sm.tile([128, 8, 128], BF16, tag="onat",
                                       name="onat0")
                    dpv = {}     # sk -> deferred (h, accs, expst) for V units

                    def V(sk):
                        def go():
                            emit_v(sk, ps_a)
                            if sk in dpv:
                                h, accs, e = dpv.pop(sk)
                                emit_pv(h, accs, sk, e)
                        return go

                    fillers = []
                    costs = []

                    def F(fn, cost):
                        fillers.append(fn)
                        costs.append(cost)

                    PC, RC, VC = 1.7, 0.25, 1.0
                    F(lambda: P(0, "q", 0), PC)      # 0
                    F(lambda: P(0, "k", 0), PC)      # 1
                    F(Rc, RC)                        # 2
                    F(lambda: P(1, "q", 0), PC)      # 3
                    F(Rc, RC)                        # 4
                    F(Rc, RC)                        # 5  mt0 ropes qb0/1 done
                    F(lambda: P(1, "k", 0), PC)      # 6
                    F(Rc, RC)                        # 7  k10
                    for sk in range(8):              # 8-15: V0-7 (+ deferred PVs)
                        F(V(sk), VC)
                    F(lambda: P(2, "k", 0), PC)      # 16
                    F(Rc, RC)                        # 17 k20
                    for sk in range(8, 12):          # 18-21
                        F(V(sk), VC)
                    F(lambda: P(3, "k", 0), PC)      # 22
                    F(Rc, RC)                        # 23 k30
                    for sk in range(12, 16):         # 24-27
                        F(V(sk), VC)
                    F(lambda: P(0, "q", 1), PC)      # 28
                    F(lambda: P(0, "k", 1), PC)      # 29
                    F(Rc, RC)                        # 30
                    F(Rc, RC)                        # 31 mt1 ropes qb0
                    F(lambda: P(1, "q", 1), PC)      # 32
                    F(lambda: P(1, "k", 1), PC)      # 33
                    F(Rc, RC)                        # 34
                    F(Rc, RC)                        # 35 mt1 ropes qb1
                    F(lambda: P(2, "q", 1), PC)      # 36
                    F(lambda: P(2, "k", 1), PC)      # 37
                    F(Rc, RC)                        # 38
                    F(Rc, RC)                        # 39
                    F(lambda: P(3, "q", 1), PC)      # 40
                    F(lambda: P(3, "k", 1), PC)      # 41
                    F(Rc, RC)                        # 42
                    F(Rc, RC)                        # 43 mt1 ropes qb2/3
                    F(lambda: P(2, "q", 0), PC)      # 44
                    F(Rc, RC)                        # 45
                    F(lambda: P(3, "q", 0), PC)      # 46
                    F(Rc, RC)                        # 47 q-mt0 qb2/3 (for qB1)

                    state = {"next": 0, "debt": 0.0}

                    def pop_to(n):
                        while state["next"] <= n:
                            fillers[state["next"]]()
                            state["debt"] -= costs[state["next"]]
                            state["next"] += 1

                    def spare(amt):
                        state["debt"] += amt
                        while (state["next"] < len(fillers)
                               and costs[state["next"]] <= state["debt"]):
                            fillers[state["next"]]()
                            state["debt"] -= costs[state["next"]]
                            state["next"] += 1

                    # prewarm the PE p-state during the initial DMA wait
                    warm_ps = ps_a.tile([128, 512], F32, tag="a",
                                        name="warmps")
                    ones_b = ones_row_r[0:1, 0:1].broadcast_to([1, 512])
                    for _ in range(10):
                        nc.tensor.matmul(warm_ps[:], ones_row_r[:], ones_b,
                                         start=True, stop=True)

                    # h0: first 8 score tiles exp'd with deferred PVs (the
                    # V projections haven't run yet)
                    pop_to(5)
                    for sk in range(4):
                        dpv[sk] = (0, accs0,
                                   emit_st_exp(0, 0, sk, tag="expst1",
                                               bufs=8, split=True))
                    pop_to(7)
                    for sk in range(4, 8):
                        dpv[sk] = (0, accs0,
                                   emit_st_exp(0, 0, sk, tag="expst1",
                                               bufs=8, split=True))
                        spare(0.45)
                    # h1's first 8 likewise (slots free as V units run)
                    h1_saved = []
                    for sk in range(8):
                        pop_to(8 + sk)
                        h1_saved.append(
                            emit_st_exp(0, 1, sk, tag="expst1", bufs=8))
                        spare(0.45)
                    # h0 second half: full chunks
                    for sk in range(8, 16):
                        pop_to(17 if sk < 12 else 23)
                        pop_to((18 + sk - 8) if sk < 12 else (24 + sk - 12))
                        emit_b_chunk(0, 0, accs0, sk, sk + 1)
                        spare(0.45)
                    emit_h_finish(0, accs0, onat0)
                    # h1: deferred PVs + remaining chunks
                    accs1 = new_accs(1)
                    for sk in range(8):
                        emit_pv(1, accs1, sk, h1_saved[sk])
                    for sk in range(8, 16):
                        pop_to(28 + (sk - 8))  # spread mt1 qb0/1 fillers
                        emit_b_chunk(0, 1, accs1, sk, sk + 1)
                        spare(0.45)
                    pop_to(35)  # mt1 qb0/1 ropes must precede head 2 scores
                    carry02 = [emit_st_exp(0, 2, k, bufs=5) for k in range(3)]
                    emit_h_finish(1, accs1, onat0)
                    emit_pair(0, 0, onat0)
                    # qB0 heads 2-3 (need the mt1 fillers)
                    onat01 = pb_sm.tile([128, 8, 128], BF16, tag="onat",
                                        name="onat01")
                    carry = carry02
                    for h in (2, 3):
                        accs = new_accs(h)
                        for sk, e in enumerate(carry):
                            emit_pv(h, accs, sk, e)
                        for sk in range(len(carry), SKT):
                            if h == 2:
                                # gradual pulls keep ACT fed at boundaries
                                pop_to(35 if sk < 4 else
                                       (min(36 + (sk - 4), 39) if sk < 8 else
                                        (min(40 + (sk - 8), 43) if sk < 12
                                         else min(44 + (sk - 12), 47))))
                            else:
                                pop_to(47)
                            emit_b_chunk(0, h, accs, sk, sk + 1)
                            spare(0.45)
                        if h == 2:
                            carry = [emit_st_exp(0, 3, k, bufs=5) for k in range(3)]
                        else:
                            pop_to(47)
                            carry = [emit_st_exp(1, 0, k, bufs=5) for k in range(3)]
                        emit_h_finish(h, accs, onat01)
                    emit_pair(0, 1, onat01)

                # ---------- qB1 + per-block O-proj ----------
                with (
                    tc.tile_pool(name="pc_y", bufs=2) as pc_y,
                    tc.tile_pool(name="ps_y", bufs=2, space="PSUM") as ps_y,
                ):
                    seq = [(1, 0), (1, 1), (1, 2), (1, 3)]
                    onats = {}
                    for idx, (qi, h) in enumerate(seq):
                        pair = (qi, h // 2)
                        if pair not in onats:
                            onats[pair] = pb_sm.tile(
                                [128, 8, 128], BF16, tag="onat",
                                name=f"onat{qi}{h//2}")
                        accs = new_accs(f"{qi}{h}")
                        for sk, e in enumerate(carry):
                            emit_pv(h, accs, sk, e)
                        emit_b_chunk(qi, h, accs, len(carry), SKT)
                        # pre-emit the next head's first scores so ACT has
                        # work across the head boundary
                        carry = []
                        if idx + 1 < len(seq):
                            nqi, nh = seq[idx + 1]
                            carry = [emit_st_exp(nqi, nh, k, bufs=5) for k in range(3)]
                        emit_h_finish(h, accs, onats[pair])
                        if h % 2 == 1:
                            emit_pair(qi, h // 2, onats[pair])
                        emit_oproj(0, h, ps_y, pc_y)
                    # tail: qB1's O-proj
                    for pair_i in range(4):
                        emit_oproj(1, pair_i, ps_y, pc_y, tail=True)

    nc.finalize()
    return nc


def _rope_tables():
    inv_freq = 1.0 / (10000.0 ** (np.arange(0, DH, 2, dtype=np.float32) / DH))
    ang = np.arange(S, dtype=np.float32)[:, None] * inv_freq[None, :]
    sin = np.concatenate([np.sin(ang), np.sin(ang)], axis=-1)  # [S, DH]
    cos = np.concatenate([np.cos(ang), np.cos(ang)], axis=-1)
    # fold the rotate-half signs into sin: rows d%64 >= 32 are negated,
    # so the rotation becomes a pure block-swap permutation
    ssin = sin.copy()
    ssin[:, DH // 2:] = -ssin[:, DH // 2:]
    sinT = np.ascontiguousarray(np.vstack([ssin.T, ssin.T]), dtype=np.float32)
    cosT = np.ascontiguousarray(np.vstack([cos.T, cos.T]), dtype=np.float32)
    return sinT, cosT  # [128, S]


def _rot_matrix():
    # pure block-swap: out[d] = u[d+32] (d%64 < 32), u[d-32] (d%64 >= 32)
    half = DH // 2
    m64 = np.zeros((DH, DH), dtype=np.float32)
    for d in range(half):
        m64[d + half, d] = 1.0
        m64[d, d + half] = 1.0
    m = np.zeros((128, 128), dtype=np.float32)
    m[0:DH, 0:DH] = m64
    m[DH:, DH:] = m64
    return m


def _make_runner(nc):
    """Build a cached jitted SPMD executor (mirrors the multi-core tail of
    concourse.bass2jax.run_bass_via_pjrt so repeat calls skip recompiles)."""
    import jax
    import numpy as _np
    from jax.sharding import Mesh, PartitionSpec
    from jax.experimental.shard_map import shard_map
    from concourse import bass2jax, mybir as _mybir

    bass2jax.install_neuronx_cc_hook()

    partition_name = (
        nc.partition_id_tensor.name if nc.partition_id_tensor else None)
    in_names, out_names, out_avals, zero_shapes = [], [], [], []
    for alloc in nc.m.functions[0].allocations:
        if not isinstance(alloc, _mybir.MemoryLocationSet):
            continue
        name = alloc.memorylocations[0].name
        if alloc.kind == "ExternalInput":
            if name != partition_name:
                in_names.append(name)
        elif alloc.kind == "ExternalOutput":
            out_names.append(name)
            shape = tuple(alloc.tensor_shape)
            dtype = _mybir.dt.np(alloc.dtype)
            out_avals.append(jax.core.ShapedArray(shape, dtype))
            zero_shapes.append((shape, dtype))
    n_params = len(in_names)
    all_names = in_names + out_names
    if partition_name is not None:
        all_names = all_names + [partition_name]

    def _body(*args):
        operands = list(args)
        if partition_name is not None:
            operands.append(bass2jax.partition_id_tensor())
        outs = bass2jax._bass_exec_p.bind(
            *operands,
            out_avals=tuple(out_avals),
            in_names=tuple(all_names),
            out_names=tuple(out_names),
            lowering_input_output_aliases=(),
            sim_require_finite=True,
            sim_require_nnan=True,
            nc=nc,
        )
        return tuple(outs)

    devices = jax.devices()[:NCORES]
    mesh = Mesh(_np.asarray(devices), ("core",))
    n_outs = len(out_names)
    sharded = jax.jit(
        shard_map(
            _body, mesh=mesh,
            in_specs=(PartitionSpec("core"),) * (n_params + n_outs),
            out_specs=(PartitionSpec("core"),) * n_outs,
            check_rep=False,
        ),
        donate_argnums=tuple(range(n_params, n_params + n_outs)),
        keep_unused=True,
    )

    def run(in_maps):
        concat_in = [
            _np.concatenate([_np.asarray(m[name]) for m in in_maps], axis=0)
            for name in in_names
        ]
        concat_zeros = [
            _np.zeros((NCORES * s[0], *s[1:]), dt) for (s, dt) in zero_shapes
        ]
        out_arrs = sharded(*concat_in, *concat_zeros)
        return [
            {
                name: _np.asarray(out_arrs[i]).reshape(
                    NCORES, *out_avals[i].shape)[c]
                for i, name in enumerate(out_names)
            }
            for c in range(NCORES)
        ]

    return run


def _get_runner():
    if "runner" not in _CACHE:
        nc = _build_nc()
        _CACHE["nc"] = nc
        _CACHE["runner"] = _make_runner(nc)
    return _CACHE["runner"]


def make_in_maps(x, wq, bq, wk, bk, wv, bv, wo, bo):
    """Build the 8 per-core input dicts from full inputs."""
    import ml_dtypes
    bf16 = ml_dtypes.bfloat16
    x = np.asarray(x, dtype=np.float32)
    if "tables" not in _CACHE:
        _CACHE["tables"] = _rope_tables()
        _CACHE["prot"] = _rot_matrix()
        _CACHE["ident"] = np.eye(128, dtype=bf16)
    sinT, cosT = _CACHE["tables"]
    prot = _CACHE["prot"]
    ident = _CACHE["ident"]
    in_maps = []
    for c in range(NCORES):
        b, hg = divmod(c, HL)
        sl = slice(hg * DHL, (hg + 1) * DHL)
        constsA = np.zeros((128, 384), np.float32)
        constsA[:, 0:128] = prot
        constsA[0, 128:128 + DHL] = np.asarray(bv, np.float32)[sl]
        constsB = np.zeros((128, 1028), np.float32)
        constsB[:, 0:2] = np.asarray(bq, np.float32)[sl].reshape(2, 128).T
        constsB[:, 2:4] = np.asarray(bk, np.float32)[sl].reshape(2, 128).T
        constsB[:, 4:516] = cosT[:, 0:512]
        constsB[:, 516:1028] = sinT[:, 0:512]
        in_maps.append({
            "xT": np.ascontiguousarray(x[b].T),
            "wq": np.ascontiguousarray(np.asarray(wq, np.float32)[:, sl]),
            "wk": np.ascontiguousarray(np.asarray(wk, np.float32)[:, sl]),
            "wv": np.ascontiguousarray(np.asarray(wv, np.float32)[:, sl]),
            "wo": np.ascontiguousarray(
                np.asarray(wo, np.float32)[sl, :].reshape(2, 128, DM)
                .transpose(1, 0, 2)).astype(bf16),
            "constsA": constsA,
            "constsB": constsB,
            "cosT": cosT,
            "sinT": sinT,
            "ident": ident,
        })
    return in_maps


def kernel(x, wq, bq, wk, bk, wv, bv, wo, bo):
    runner = _get_runner()
    in_maps = make_in_maps(x, wq, bq, wk, bk, wv, bv, wo, bo)
    results = runner(in_maps)
    bo = np.asarray(bo, dtype=np.float32)
    out = np.empty((B, S, DM), dtype=np.float32)
    for b in range(B):
        acc = results[b * HL + 0]["y"].astype(np.float32)
        for hg in range(1, HL):
            acc += results[b * HL + hg]["y"].astype(np.float32)
        out[b] = acc + bo[None, :]
    return out


# revision 45
# speedup vs baseline: 1.1829x; 1.0031x over previous
"""MultiHeadAttention + RoPE kernel for 8 Trainium2 NeuronCores.

Sharding: core c in 0..7 -> batch b = c//4, head-group hg = c%4 (4 heads
each).  Each core computes its 4 heads' attention for its batch and a
partial output projection y_part = out_heads @ wo[head rows]; the host
sums the 4 partials per batch (bf16 -> f32) and adds bo.

Per-core dataflow:
  - x arrives in four 512-query column-block DMAs; Q/K projections
    (fp32r, transposed [depth, S]) and V (natural, bf16 + ones column)
    are paced into the attention pipeline by a debt-based filler
    scheduler so the ACT engine's exp stream starts early and PE
    rarely idles
  - RoPE: dest = (acc+b)*cos + blockswap((acc+b)*sin_signed), where the
    sign of sin is folded into the host table so the rotation is a pure
    permutation matmul
  - scores transposed: matmul(lhsT=KT tile, rhs=QT) -> ST [128 keys,
    2x512 queries]; exp on ACT (scale=1/8), output bf16
  - PV natural: exp'd scores are the STATIONARY operand (ldweights is
    free), V' [128,65] bf16 moving -> out_nat [128 q, 65] accumulated
    over 16 key tiles in PSUM; col 64 = softmax denominator.  The four
    per-bank accumulators share one PSUM zero-region, so a whole-bank
    zero-write matmul opens each accumulation epoch and the PV matmuls
    run with start=False/skip_group_check
  - normalize: DVE reciprocal + per-partition scale during PSUM evac
    (bf16); head pairs assembled to [128 q, 128 d] and PE-transposed
    to [128 d, q] for the O-projection (gpsimd cannot touch PSUM, so
    all PSUM-side elementwise work lives on DVE/ACT)
  - O-proj per query block: matmul(lhsT=outT pair tile, rhs=wo bf16),
    y stored bf16 via paired-row DMAs; host sums partials + bo in f32
"""

import numpy as np

import concourse.bacc as bacc
import concourse.mybir as mybir
from concourse.tile import TileContext

try:  # persistent XLA compile cache: repeat processes skip the ~4min compile
    import jax as _jax
    _jax.config.update("jax_compilation_cache_dir", "/tmp/jax_comp_cache")
    _jax.config.update("jax_persistent_cache_min_compile_time_secs", 1.0)
except Exception:
    pass

B, S, DM, H, DH = 2, 2048, 1024, 16, 64
NCORES = 8
HL = 4                 # heads per core
DHL = HL * DH          # 256
KCH = DM // 128        # 8 k-chunks of the model-dim contraction
SKT = S // 128         # 16 key tiles
QB = 1024              # phase-B query block
NQB = S // QB          # 2

F32 = mybir.dt.float32
F32R = mybir.dt.float32r
BF16 = mybir.dt.bfloat16
EXP = mybir.ActivationFunctionType.Exp
COPY = mybir.ActivationFunctionType.Copy
ADD = mybir.AluOpType.add
MULT = mybir.AluOpType.mult

_CACHE = {}


def _build_nc():
    nc = bacc.Bacc()
    xT = nc.dram_tensor("xT", [DM, S], F32R, kind="ExternalInput")
    wq = nc.dram_tensor("wq", [DM, DHL], F32R, kind="ExternalInput")
    wk = nc.dram_tensor("wk", [DM, DHL], F32R, kind="ExternalInput")
    wv = nc.dram_tensor("wv", [DM, DHL], F32R, kind="ExternalInput")
    wo = nc.dram_tensor("wo", [128, 2, DM], BF16, kind="ExternalInput")
    cosT = nc.dram_tensor("cosT", [128, S], F32, kind="ExternalInput")
    sinT = nc.dram_tensor("sinT", [128, S], F32, kind="ExternalInput")
    # constsA (f32r): prot [0:128], bv row0 [128:384]
    # constsB (f32): bq [0:2], bk [2:4], cos qb0 [4:516], sin qb0 [516:1028]
    constsA = nc.dram_tensor("constsA", [128, 384], F32R,
                             kind="ExternalInput")
    constsB = nc.dram_tensor("constsB", [128, 1028], F32,
                             kind="ExternalInput")
    ident = nc.dram_tensor("ident", [128, 128], BF16, kind="ExternalInput")
    y = nc.dram_tensor("y", [S, DM], BF16, kind="ExternalOutput")
    yre = y.rearrange("(a p) n -> p a n", p=128)

    with TileContext(nc) as tc:
        with tc.tile_pool(name="p0", bufs=1) as p0:
            # persistent SBUF
            qrope_r = p0.tile([128, 2, S], F32R)
            krope_r = p0.tile([128, 2, S], F32R)
            v_r = p0.tile([128, SKT, HL, DH + 1], BF16)
            outT_sb = p0.tile([128, 2, S], BF16)
            xT_r = p0.tile([128, KCH, S], F32R)
            wq_r = p0.tile([128, KCH, DHL], F32R)
            wk_r = p0.tile([128, KCH, DHL], F32R)
            wv_r = p0.tile([128, KCH, DHL], F32R)
            wo_r = p0.tile([128, 2, DM], BF16)
            cos_sb = p0.tile([128, S], F32)
            sin_sb = p0.tile([128, S], F32)
            cA = p0.tile([128, 384], F32R)
            cB = p0.tile([128, 1028], F32)
            ident_r = p0.tile([128, 128], BF16)
            ones_row_r = p0.tile([1, 128], F32R)
            zrow = p0.tile([1, 512], BF16)
            warm = p0.tile([1, 128], F32)
            prot_r = cA[:, 0:128]
            bv_r = cA[0:1, 128:128 + DHL]
            bq_sb = cB[:, 0:2]
            bk_sb = cB[:, 2:4]

            def load_w(dst, src):
                nc.sync.dma_start(
                    dst[:], src.rearrange("(k p) n -> p k n", p=128))

            def load_xqb(qb, cs_first=False):
                q0 = qb * 512

                def cs():
                    if qb > 0:
                        nc.sync.dma_start(cos_sb[:, q0:q0 + 512],
                                          cosT[:, q0:q0 + 512])
                        nc.sync.dma_start(sin_sb[:, q0:q0 + 512],
                                          sinT[:, q0:q0 + 512])
                src = xT[:, q0:q0 + 512].rearrange("(k p) n -> p k n", p=128)
                if cs_first:
                    cs()
                nc.sync.dma_start(xT_r[:, :, q0:q0 + 512], src)
                if not cs_first:
                    cs()

            nc.sync.dma_start(cB[:], constsB[:, :])
            nc.sync.dma_start(cA[:], constsA[:, :])
            load_w(wq_r, wq)
            load_xqb(0)
            load_w(wk_r, wk)
            nc.vector.memset(warm[:], 1.0)
            nc.vector.tensor_copy(ones_row_r[:], warm[:])
            nc.vector.memset(zrow[:], 0.0)
            nc.vector.memset(v_r[:, :, :, DH:DH + 1], 1.0)
            # preload the exp ACT table while ACT is idle
            nc.scalar.activation(warm[:], warm[:], EXP, scale=0.125)
            load_xqb(1, cs_first=True)
            load_w(wv_r, wv)
            load_xqb(2)
            nc.sync.dma_start(ident_r[:], ident[:, :])
            load_xqb(3)
            nc.sync.dma_start(wo_r[:], wo[:, :, :])

            with (
                tc.tile_pool(name="pb_exp", bufs=3) as pb_exp,
                tc.tile_pool(name="pb_sm", bufs=2) as pb_sm,
                tc.tile_pool(name="ps_b", bufs=2, space="PSUM") as ps_b,
            ):
                # ---------- emit helpers ----------
                def emit_proj_acc(qb, w_r, b_sb, dest, mt, ps_a, pa_t,
                                  eng=None):
                    """Projection accumulation + rope multiplies; returns a
                    closure emitting the rotation matmul + final add, to be
                    placed a couple of PE units later in the stream.  The
                    elementwise rope ops run on `eng` (DVE for Q, Pool for K
                    so the two chains run in parallel early on)."""
                    q0 = qb * 512
                    eng = eng or nc.vector
                    acc = ps_a.tile([128, 512], F32, tag="a", name="acc")
                    for c in range(KCH):
                        nc.tensor.matmul(
                            acc[:], w_r[:, c, mt * 128:(mt + 1) * 128],
                            xT_r[:, c, q0:q0 + 512],
                            start=(c == 0), stop=(c == KCH - 1))
                    cos_src = (cB[:, 4:516] if qb == 0
                               else cos_sb[:, q0:q0 + 512])
                    sin_src = (cB[:, 516:1028] if qb == 0
                               else sin_sb[:, q0:q0 + 512])
                    u = pa_t.tile([128, 512], F32R, tag="u")
                    eng.scalar_tensor_tensor(
                        out=u[:], in0=acc[:], scalar=b_sb[:, mt:mt + 1],
                        in1=sin_src, op0=ADD, op1=MULT)
                    t1 = pa_t.tile([128, 512], F32, tag="t1")
                    eng.scalar_tensor_tensor(
                        out=t1[:], in0=acc[:], scalar=b_sb[:, mt:mt + 1],
                        in1=cos_src, op0=ADD, op1=MULT)

                    def finish_rot():
                        rot = ps_a.tile([128, 512], F32, tag="a", name="rot")
                        nc.tensor.matmul(rot[:], prot_r[:, :], u[:],
                                         start=True, stop=True)
                        eng.tensor_add(dest[:, mt, q0:q0 + 512],
                                       t1[:], rot[:])
                    return finish_rot

                def emit_v(sk, ps_a):
                    vps = ps_a.tile([128, 512], F32, tag="a", name="vps")
                    for c in range(KCH):
                        nc.tensor.matmul(
                            vps[:, 0:DHL],
                            xT_r[:, c, sk * 128:(sk + 1) * 128],
                            wv_r[:, c, :], start=(c == 0), stop=False)
                    nc.tensor.matmul(vps[:, 0:DHL], ones_row_r[:], bv_r[:],
                                     start=False, stop=True)
                    nc.gpsimd.tensor_copy(
                        v_r[:, sk, :, 0:DH],
                        vps[:, 0:DHL].rearrange("p (h d) -> p h d", h=HL))

                def emit_st_exp(qi, h, sk, tag="expst", bufs=5,
                                split=False):
                    q0 = qi * QB
                    mt = h // 2
                    half = (h % 2) * DH
                    qt_h = qrope_r[half:half + DH, mt, :]
                    kt_h = krope_r[half:half + DH, mt, :]
                    st = ps_b.tile([128, 2, 512], F32, tag="st", name="st")
                    expst = pb_exp.tile([128, 2, 512], BF16, tag=tag,
                                        bufs=bufs, name="expst")
                    for n in range(2):
                        nc.tensor.matmul(
                            st[:, n, :],
                            kt_h[:, sk * 128:(sk + 1) * 128],
                            qt_h[:, q0 + n * 512:q0 + (n + 1) * 512],
                            start=True, stop=True)
                        if split:
                            nc.scalar.activation(expst[:, n, :], st[:, n, :],
                                                 EXP, scale=0.125)
                    if not split:
                        nc.scalar.activation(expst[:], st[:], EXP, scale=0.125)
                    return expst

                def emit_pv(h, accs, sk, expst):
                    # accumulators share PSUM banks, so groups are managed
                    # by the whole-bank clear matmul in new_accs
                    for qt in range(8):
                        nc.tensor.matmul(
                            accs[qt // 4][:, qt % 4, 0:DH + 1],
                            expst[:, qt // 4,
                                  (qt % 4) * 128:(qt % 4 + 1) * 128],
                            v_r[:, sk, h, :],
                            start=False, stop=False, skip_group_check=True)

                def emit_b_chunk(qi, h, accs, sk_lo, sk_hi):
                    for sk in range(sk_lo, sk_hi):
                        expst = emit_st_exp(qi, h, sk)
                        emit_pv(h, accs, sk, expst)

                def emit_h_finish(h, accs, onat):
                    half = (h % 2) * DH
                    rec = pb_sm.tile([128, 8, 1], F32, tag="rec")
                    for g in range(2):
                        nc.vector.reciprocal(rec[:, g * 4:(g + 1) * 4, :],
                                             accs[g][:, :, DH:DH + 1])
                        for qt in range(g * 4, g * 4 + 4):
                            eng = nc.vector if qt % 2 == 0 else nc.gpsimd
                            eng.tensor_scalar(
                                out=onat[:, qt, half:half + DH],
                                in0=accs[qt // 4][:, qt % 4, 0:DH],
                                scalar1=rec[:, qt, :],
                                scalar2=None, op0=MULT)

                def emit_pair(qi, p, onat):
                    q0 = qi * QB
                    for qt in range(8):
                        pt = ps_b.tile([128, 128], BF16, tag="out", name="pt")
                        nc.tensor.transpose(pt[:], onat[:, qt, :], ident_r[:])
                        nc.vector.tensor_copy(
                            outT_sb[:, p, q0 + qt * 128:q0 + (qt + 1) * 128],
                            pt[:])

                def emit_oproj(qi, pair_i, ps_y, pc_y, tail=False):
                    # one pair of query tiles -> one y DMA
                    ysb = pc_y.tile([128, 2, DM], BF16, tag="ysb", bufs=3)
                    for j in range(2):
                        qt = qi * 8 + pair_i * 2 + j
                        for oc in range(2):
                            yp = ps_y.tile([128, 512], F32, tag="y")
                            for p in range(2):
                                nc.tensor.matmul(
                                    yp[:],
                                    outT_sb[:, p, qt * 128:(qt + 1) * 128],
                                    wo_r[:, p, oc * 512:(oc + 1) * 512],
                                    start=(p == 0), stop=(p == 1))
                            if tail and oc == 0:
                                # ACT is idle at the tail; use it for evac
                                nc.scalar.activation(
                                    ysb[:, j, oc * 512:(oc + 1) * 512],
                                    yp[:], COPY)
                            else:
                                eng = nc.vector if oc == 0 else nc.gpsimd
                                eng.tensor_copy(
                                    ysb[:, j, oc * 512:(oc + 1) * 512], yp[:])
                    qp = qi * 4 + pair_i
                    if tail and pair_i == 3:
                        # split the last store so the final (smaller) DMA
                        # starts earlier and the end barrier waits less
                        nc.sync.dma_start(yre[:, 2 * qp:2 * qp + 1, :],
                                          ysb[:, 0:1, :])
                        nc.sync.dma_start(yre[:, 2 * qp + 1:2 * qp + 2, :],
                                          ysb[:, 1:2, :])
                    else:
                        nc.sync.dma_start(yre[:, 2 * qp:2 * qp + 2, :],
                                          ysb[:])

                def new_accs(tag_i):
                    accs = [ps_b.tile([128, 4, 128], F32, tag="out",
                                      name=f"acc{tag_i}{g}") for g in range(2)]
                    for a in accs:
                        # whole-bank zero-write opens the accumulation epoch
                        # and orders (WAW) ahead of every PV matmul
                        nc.tensor.matmul(a[:, :, :], zrow[0:1, 0:128],
                                         zrow[:], start=True, stop=True)
                    return accs

                # ---------- phase A interleaved with head 0 of qB0 ----------
                with (
                    tc.tile_pool(name="pa_t", bufs=3) as pa_t,
                    tc.tile_pool(name="ps_a", bufs=2, space="PSUM") as ps_a,
                ):
                    pending = []

                    def P(qb, w, mt):
                        wr, bs, dst, eng = ((wq_r, bq_sb, qrope_r, nc.vector)
                                            if w == "q" else
                                            (wk_r, bk_sb, krope_r, nc.gpsimd))
                        pending.append(
                            emit_proj_acc(qb, wr, bs, dst, mt, ps_a, pa_t,
                                          eng))

                    def Rc():
                        pending.pop(0)()

                    # --- filler schedule: phase A work paced into the gaps
                    # of the attention pipeline (PE is the global bottleneck;
                    # ACT-bound stretches leave ~0.35us/exp of PE spare) ---
                    accs0 = new_accs(0)
                    onat0 = pb_sm.tile([128, 8, 128], BF16, tag="onat",
                                       name="onat0")
                    dpv = {}     # sk -> deferred (h, accs, expst) for V units

                    def V(sk):
                        def go():
                            emit_v(sk, ps_a)
                            if sk in dpv:
                                h, accs, e = dpv.pop(sk)
                                emit_pv(h, accs, sk, e)
                        return go

                    fillers = []
                    costs = []

                    def F(fn, cost):
                        fillers.append(fn)
                        costs.append(cost)

                    PC, RC, VC = 1.7, 0.25, 1.0
                    F(lambda: P(0, "q", 0), PC)      # 0
                    F(lambda: P(0, "k", 0), PC)      # 1
                    F(Rc, RC)                        # 2
                    F(lambda: P(1, "q", 0), PC)      # 3
                    F(Rc, RC)                        # 4
                    F(Rc, RC)                        # 5  mt0 ropes qb0/1 done
                    F(lambda: P(1, "k", 0), PC)      # 6
                    F(Rc, RC)                        # 7  k10
                    for sk in range(8):              # 8-15: V0-7 (+ deferred PVs)
                        F(V(sk), VC)
                    F(lambda: P(2, "k", 0), PC)      # 16
                    F(Rc, RC)                        # 17 k20
                    for sk in range(8, 12):          # 18-21
                        F(V(sk), VC)
                    F(lambda: P(3, "k", 0), PC)      # 22
                    F(Rc, RC)                        # 23 k30
                    for sk in range(12, 16):         # 24-27
                        F(V(sk), VC)
                    F(lambda: P(0, "q", 1), PC)      # 28
                    F(lambda: P(0, "k", 1), PC)      # 29
                    F(Rc, RC)                        # 30
                    F(Rc, RC)                        # 31 mt1 ropes qb0
                    F(lambda: P(1, "q", 1), PC)      # 32
                    F(lambda: P(1, "k", 1), PC)      # 33
                    F(Rc, RC)                        # 34
                    F(Rc, RC)                        # 35 mt1 ropes qb1
                    F(lambda: P(2, "q", 1), PC)      # 36
                    F(lambda: P(2, "k", 1), PC)      # 37
                    F(Rc, RC)                        # 38
                    F(Rc, RC)                        # 39
                    F(lambda: P(3, "q", 1), PC)      # 40
                    F(lambda: P(3, "k", 1), PC)      # 41
                    F(Rc, RC)                        # 42
                    F(Rc, RC)                        # 43 mt1 ropes qb2/3
                    F(lambda: P(2, "q", 0), PC)      # 44
                    F(Rc, RC)                        # 45
                    F(lambda: P(3, "q", 0), PC)      # 46
                    F(Rc, RC)                        # 47 q-mt0 qb2/3 (for qB1)

                    state = {"next": 0, "debt": 0.0}

                    def pop_to(n):
                        while state["next"] <= n:
                            fillers[state["next"]]()
                            state["debt"] -= costs[state["next"]]
                            state["next"] += 1

                    def spare(amt):
                        state["debt"] += amt
                        while (state["next"] < len(fillers)
                               and costs[state["next"]] <= state["debt"]):
                            fillers[state["next"]]()
                            state["debt"] -= costs[state["next"]]
                            state["next"] += 1

                    # prewarm the PE p-state during the initial DMA wait
                    warm_ps = ps_a.tile([128, 512], F32, tag="a",
                                        name="warmps")
                    ones_b = ones_row_r[0:1, 0:1].broadcast_to([1, 512])
                    for _ in range(10):
                        nc.tensor.matmul(warm_ps[:], ones_row_r[:], ones_b,
                                         start=True, stop=True)

                    # h0: first 8 score tiles exp'd with deferred PVs (the
                    # V projections haven't run yet)
                    pop_to(5)
                    for sk in range(4):
                        dpv[sk] = (0, accs0,
                                   emit_st_exp(0, 0, sk, tag="expst1",
                                               bufs=8, split=True))
                    pop_to(7)
                    for sk in range(4, 8):
                        dpv[sk] = (0, accs0,
                                   emit_st_exp(0, 0, sk, tag="expst1",
                                               bufs=8, split=True))
                        spare(0.45)
                    # h1's first 8 likewise (slots free as V units run)
                    h1_saved = []
                    for sk in range(8):
                        pop_to(8 + sk)
                        h1_saved.append(
                            emit_st_exp(0, 1, sk, tag="expst1", bufs=8))
                        spare(0.45)
                    # h0 second half: full chunks
                    for sk in range(8, 16):
                        pop_to(17 if sk < 12 else 23)
                        pop_to((18 + sk - 8) if sk < 12 else (24 + sk - 12))
                        emit_b_chunk(0, 0, accs0, sk, sk + 1)
                        spare(0.45)
                    emit_h_finish(0, accs0, onat0)
                    # h1: deferred PVs + remaining chunks
                    accs1 = new_accs(1)
                    for sk in range(8):
                        emit_pv(1, accs1, sk, h1_saved[sk])
                    for sk in range(8, 16):
                        pop_to(28 + (sk - 8))  # spread mt1 qb0/1 fillers
                        emit_b_chunk(0, 1, accs1, sk, sk + 1)
                        spare(0.45)
                    pop_to(35)  # mt1 qb0/1 ropes must precede head 2 scores
                    carry02 = [emit_st_exp(0, 2, k, bufs=5) for k in range(3)]
                    emit_h_finish(1, accs1, onat0)
                    emit_pair(0, 0, onat0)
                    # qB0 heads 2-3 (need the mt1 fillers)
                    onat01 = pb_sm.tile([128, 8, 128], BF16, tag="onat",
                                        name="onat01")
                    carry = carry02
                    for h in (2, 3):
                        accs = new_accs(h)
                        for sk, e in enumerate(carry):
                            emit_pv(h, accs, sk, e)
                        for sk in range(len(carry), SKT):
                            if h == 2:
                                # gradual pulls keep ACT fed at boundaries
                                pop_to(35 if sk < 4 else
                                       (min(36 + (sk - 4), 39) if sk < 8 else
                                        (min(40 + (sk - 8), 43) if sk < 12
                                         else min(44 + (sk - 12), 47))))
                            else:
                                pop_to(47)
                            emit_b_chunk(0, h, accs, sk, sk + 1)
                            spare(0.45)
                        if h == 2:
                            carry = [emit_st_exp(0, 3, k, bufs=5) for k in range(3)]
                        else:
                            pop_to(47)
                            carry = [emit_st_exp(1, 0, k, bufs=5) for k in range(3)]
                        emit_h_finish(h, accs, onat01)
                    emit_pair(0, 1, onat01)

                # ---------- qB1 + per-block O-proj ----------
                with (
                    tc.tile_pool(name="pc_y", bufs=2) as pc_y,
                    tc.tile_pool(name="ps_y", bufs=2, space="PSUM") as ps_y,
                ):
                    seq = [(1, 0), (1, 1), (1, 2), (1, 3)]
                    onats = {}
                    for idx, (qi, h) in enumerate(seq):
                        pair = (qi, h // 2)
                        if pair not in onats:
                            onats[pair] = pb_sm.tile(
                                [128, 8, 128], BF16, tag="onat",
                                name=f"onat{qi}{h//2}")
                        accs = new_accs(f"{qi}{h}")
                        for sk, e in enumerate(carry):
                            emit_pv(h, accs, sk, e)
                        emit_b_chunk(qi, h, accs, len(carry), SKT)
                        # pre-emit the next head's first scores so ACT has
                        # work across the head boundary
                        carry = []
                        if idx + 1 < len(seq):
                            nqi, nh = seq[idx + 1]
                            carry = [emit_st_exp(nqi, nh, k, bufs=5) for k in range(3)]
                        emit_h_finish(h, accs, onats[pair])
                        if h % 2 == 1:
                            emit_pair(qi, h // 2, onats[pair])
                        emit_oproj(0, h, ps_y, pc_y)
                    # tail: qB1's O-proj
                    for pair_i in range(4):
                        emit_oproj(1, pair_i, ps_y, pc_y, tail=True)

    nc.finalize()
    return nc


def _rope_tables():
    inv_freq = 1.0 / (10000.0 ** (np.arange(0, DH, 2, dtype=np.float32) / DH))
    ang = np.arange(S, dtype=np.float32)[:, None] * inv_freq[None, :]
    sin = np.concatenate([np.sin(ang), np.sin(ang)], axis=-1)  # [S, DH]
    cos = np.concatenate([np.cos(ang), np.cos(ang)], axis=-1)
    # fold the rotate-half signs into sin: rows d%64 >= 32 are negated,
    # so the rotation becomes a pure block-swap permutation
    ssin = sin.copy()
    ssin[:, DH // 2:] = -ssin[:, DH // 2:]
    sinT = np.ascontiguousarray(np.vstack([ssin.T, ssin.T]), dtype=np.float32)
    cosT = np.ascontiguousarray(np.vstack([cos.T, cos.T]), dtype=np.float32)
    return sinT, cosT  # [128, S]


def _rot_matrix():
    # pure block-swap: out[d] = u[d+32] (d%64 < 32), u[d-32] (d%64 >= 32)
    half = DH // 2
    m64 = np.zeros((DH, DH), dtype=np.float32)
    for d in range(half):
        m64[d + half, d] = 1.0
        m64[d, d + half] = 1.0
    m = np.zeros((128, 128), dtype=np.float32)
    m[0:DH, 0:DH] = m64
    m[DH:, DH:] = m64
    return m


def _make_runner(nc):
    """Build a cached jitted SPMD executor (mirrors the multi-core tail of
    concourse.bass2jax.run_bass_via_pjrt so repeat calls skip recompiles)."""
    import jax
    import numpy as _np
    from jax.sharding import Mesh, PartitionSpec
    from jax.experimental.shard_map import shard_map
    from concourse import bass2jax, mybir as _mybir

    bass2jax.install_neuronx_cc_hook()

    partition_name = (
        nc.partition_id_tensor.name if nc.partition_id_tensor else None)
    in_names, out_names, out_avals, zero_shapes = [], [], [], []
    for alloc in nc.m.functions[0].allocations:
        if not isinstance(alloc, _mybir.MemoryLocationSet):
            continue
        name = alloc.memorylocations[0].name
        if alloc.kind == "ExternalInput":
            if name != partition_name:
                in_names.append(name)
        elif alloc.kind == "ExternalOutput":
            out_names.append(name)
            shape = tuple(alloc.tensor_shape)
            dtype = _mybir.dt.np(alloc.dtype)
            out_avals.append(jax.core.ShapedArray(shape, dtype))
            zero_shapes.append((shape, dtype))
    n_params = len(in_names)
    all_names = in_names + out_names
    if partition_name is not None:
        all_names = all_names + [partition_name]

    def _body(*args):
        operands = list(args)
        if partition_name is not None:
            operands.append(bass2jax.partition_id_tensor())
        outs = bass2jax._bass_exec_p.bind(
            *operands,
            out_avals=tuple(out_avals),
            in_names=tuple(all_names),
            out_names=tuple(out_names),
            lowering_input_output_aliases=(),
            sim_require_finite=True,
            sim_require_nnan=True,
            nc=nc,
        )
        return tuple(outs)

    devices = jax.devices()[:NCORES]
    mesh = Mesh(_np.asarray(devices), ("core",))
    n_outs = len(out_names)
    sharded = jax.jit(
        shard_map(
            _body, mesh=mesh,
            in_specs=(PartitionSpec("core"),) * (n_params + n_outs),
            out_specs=(PartitionSpec("core"),) * n_outs,
            check_rep=False,
        ),
        donate_argnums=tuple(range(n_params, n_params + n_outs)),
        keep_unused=True,
    )

    def run(in_maps):
        concat_in = [
            _np.concatenate([_np.asarray(m[name]) for m in in_maps], axis=0)
            for name in in_names
        ]
        concat_zeros = [
            _np.zeros((NCORES * s[0], *s[1:]), dt) for (s, dt) in zero_shapes
        ]
        out_arrs = sharded(*concat_in, *concat_zeros)
        return [
            {
                name: _np.asarray(out_arrs[i]).reshape(
                    NCORES, *out_avals[i].shape)[c]
                for i, name in enumerate(out_names)
            }
            for c in range(NCORES)
        ]

    return run


def _get_runner():
    if "runner" not in _CACHE:
        nc = _build_nc()
        _CACHE["nc"] = nc
        _CACHE["runner"] = _make_runner(nc)
    return _CACHE["runner"]


def make_in_maps(x, wq, bq, wk, bk, wv, bv, wo, bo):
    """Build the 8 per-core input dicts from full inputs."""
    import ml_dtypes
    bf16 = ml_dtypes.bfloat16
    x = np.asarray(x, dtype=np.float32)
    if "tables" not in _CACHE:
        _CACHE["tables"] = _rope_tables()
        _CACHE["prot"] = _rot_matrix()
        _CACHE["ident"] = np.eye(128, dtype=bf16)
    sinT, cosT = _CACHE["tables"]
    prot = _CACHE["prot"]
    ident = _CACHE["ident"]
    in_maps = []
    for c in range(NCORES):
        b, hg = divmod(c, HL)
        sl = slice(hg * DHL, (hg + 1) * DHL)
        constsA = np.zeros((128, 384), np.float32)
        constsA[:, 0:128] = prot
        constsA[0, 128:128 + DHL] = np.asarray(bv, np.float32)[sl]
        constsB = np.zeros((128, 1028), np.float32)
        constsB[:, 0:2] = np.asarray(bq, np.float32)[sl].reshape(2, 128).T
        constsB[:, 2:4] = np.asarray(bk, np.float32)[sl].reshape(2, 128).T
        constsB[:, 4:516] = cosT[:, 0:512]
        constsB[:, 516:1028] = sinT[:, 0:512]
        in_maps.append({
            "xT": np.ascontiguousarray(x[b].T),
            "wq": np.ascontiguousarray(np.asarray(wq, np.float32)[:, sl]),
            "wk": np.ascontiguousarray(np.asarray(wk, np.float32)[:, sl]),
            "wv": np.ascontiguousarray(np.asarray(wv, np.float32)[:, sl]),
            "wo": np.ascontiguousarray(
                np.asarray(wo, np.float32)[sl, :].reshape(2, 128, DM)
                .transpose(1, 0, 2)).astype(bf16),
            "constsA": constsA,
            "constsB": constsB,
            "cosT": cosT,
            "sinT": sinT,
            "ident": ident,
        })
    return in_maps


def kernel(x, wq, bq, wk, bk, wv, bv, wo, bo):
    runner = _get_runner()
    in_maps = make_in_maps(x, wq, bq, wk, bk, wv, bv, wo, bo)
    results = runner(in_maps)
    bo = np.asarray(bo, dtype=np.float32)
    out = np.empty((B, S, DM), dtype=np.float32)
    for b in range(B):
        acc = results[b * HL + 0]["y"].astype(np.float32)
        for hg in range(1, HL):
            acc += results[b * HL + hg]["y"].astype(np.float32)
        out[b] = acc + bo[None, :]
    return out
